# revision 31
# baseline (speedup 1.0000x reference)
"""Trainium2 Bass kernel for the DocRED-style segment_reduce model.

Sharding: 8 cores, data-parallel: core c -> (doc = c//2, pair-half = c%2).
Each core independently computes logits for its 256 pairs. No collectives.
All segment reductions / gathers are lowered to one-hot matmuls whose
one-hot matrices are built on the host from the integer inputs and passed
as per-core input tensors (the SPMD program itself is index-agnostic).

Key facts driving the design (measured on HW):
  - the PE throttles to ~50% utilization under sustained load
    (throttle_activity_1_avg_util_limit=0.5), so total PE cycles are the
    main budget; short bursts run at 2.4 GHz, sustained streams ~1.2 GHz
  - DVE/GPSIMD can read at most one (DVE) / zero (GPSIMD) PSUM operands,
    so every PE result used by a product must be drained by ACT first
  - the attention-gather one-hots are EXACT 0/1 (or 1/64) matrices: the
    segment-mean scale cancels in the rel normalization, so the gather
    matmuls can run in fp8e4m3 DoubleRow (2x PE throughput) with only
    the attention values themselves quantized (DOCRED_P3_FP8=1)
"""

import os

import numpy as np

import concourse.bacc as bacc
import concourse.bass as bass
import concourse.mybir as mybir
import concourse.tile as tile
from concourse.bass_utils import run_bass_kernel_spmd

B, M, H = 4, 128, 1024
NH, L = 16, 1024
E, R = 64, 512
EMB, BS, NCL = 768, 64, 97
K12 = EMB // BS  # 12 blocks
NCORES = 8
RPC = R // 2  # pairs per core

F32 = mybir.dt.float32
F16 = mybir.dt.float16
BF16 = mybir.dt.bfloat16
FP8 = mybir.dt.float8e4

MM_MODE = os.environ.get("DOCRED_MM_MODE", "f16")
# P3 gather matmuls in fp8e4m3 DoubleRow (2x PE throughput)
P3_FP8 = os.environ.get("DOCRED_P3_FP8", "0") == "1"
# P3 per-pack product route ("a": ACT copies x-half, DVE mults SBUF x PSUM;
# "b": ACT copies both, DVE fp16; "g": ACT copies both, GpSimd mults)
P3_ROUTES = os.environ.get("DOCRED_P3_ROUTES", "a,a,a,g").split(",")
# P6 k-block routes ("pa": PE one-hot replication + ACT copy (even k only);
# "pd": PE replication, DVE mults from PSUM; "d": DMA broadcast replication)
P6_ROUTES = os.environ.get(
    "DOCRED_P6_ROUTES", "pa,d,d,d,pa,d,d,d,d,d,d,d").split(",")

_FULL_KEY = (MM_MODE, P3_FP8, tuple(P3_ROUTES), tuple(P6_ROUTES))


def _fdt():
    return BF16 if MM_MODE == "bf16" else F16


def _np_fdt():
    import ml_dtypes

    return np.dtype(ml_dtypes.bfloat16) if MM_MODE == "bf16" else np.float16


def _np_fp8():
    import ml_dtypes

    return np.dtype(ml_dtypes.float8_e4m3)


class _Builder:
    def __init__(self):
        self.fdt = _fdt()
        nc = bacc.Bacc("TRN2", target_bir_lowering=False, debug=False)
        self.nc = nc
        fdt = self.fdt
        d = {}
        d["ent"] = nc.dram_tensor("ent", [M, H], F32, kind="ExternalInput")
        if P3_FP8:
            # [k(64), t(2), lc, h, 128] fp8 (m = t*64 + k)
            d["attn"] = nc.dram_tensor("attn", [64, 2 * 8 * NH * 128], FP8,
                                       kind="ExternalInput")
            d["ohxy2"] = nc.dram_tensor("ohxy2", [64, 2 * 2 * RPC], FP8,
                                        kind="ExternalInput")
        else:
            d["attn"] = nc.dram_tensor("attn", [M, 8 * NH * 128], fdt,
                                       kind="ExternalInput")
            d["ohxy2"] = nc.dram_tensor("ohxy2", [M, 2 * RPC], fdt,
                                        kind="ExternalInput")
        d["epsb"] = nc.dram_tensor("epsb", [128, 2], F32, kind="ExternalInput")
        d["seq"] = nc.dram_tensor("seq", [128, 8 * (L + 1)], fdt,
                                  kind="ExternalInput")
        d["ssum"] = nc.dram_tensor("ssum", [M, E], fdt, kind="ExternalInput")
        d["eadd"] = nc.dram_tensor("eadd", [E, 1], F32, kind="ExternalInput")
        d["ohx"] = nc.dram_tensor("ohx", [E, RPC], fdt, kind="ExternalInput")
        d["ohy"] = nc.dram_tensor("ohy", [E, RPC], fdt, kind="ExternalInput")
        d["wh"] = nc.dram_tensor("wh", [128, 16 * EMB], fdt, kind="ExternalInput")
        d["wt"] = nc.dram_tensor("wt", [128, 16 * EMB], fdt, kind="ExternalInput")
        d["bh"] = nc.dram_tensor("bh", [128, EMB // 128], F32, kind="ExternalInput")
        d["bt"] = nc.dram_tensor("bt", [128, EMB // 128], F32, kind="ExternalInput")
        d["wb"] = nc.dram_tensor("wb", [128, 384 * NCL], fdt, kind="ExternalInput")
        d["bbc"] = nc.dram_tensor("bbc", [NCL, 1], F32, kind="ExternalInput")
        d["ident"] = nc.dram_tensor("ident", [128, 128], fdt, kind="ExternalInput")
        d["repm"] = nc.dram_tensor("repm", [E, 32 * 128], fdt, kind="ExternalInput")
        d["lt"] = nc.dram_tensor("lt", [NCL, RPC], F32, kind="ExternalOutput")
        self.d = d
        with tile.TileContext(nc) as tc:
            self.build(tc)
        nc.compile()

    def mm(self, out, lhsT, rhs, **kw):
        return self.nc.tensor.matmul(out, lhsT, rhs, **kw)

    def tp(self, out, in_, ident, **kw):
        return self.nc.tensor.matmul(out, in_, ident, is_transpose=True, **kw)

    def build(self, tc):
        nc = self.nc
        d = self.d
        fdt = self.fdt
        AF = mybir.ActivationFunctionType
        DR = mybir.MatmulPerfMode.DoubleRow

        with (
            tc.tile_pool(name="pin", bufs=1) as pin,
            tc.tile_pool(name="mid", bufs=1) as mid,
            tc.tile_pool(name="prodp", bufs=2) as prodp,
            tc.tile_pool(name="dramp", bufs=1, space="DRAM") as dramp,
        ):
            # phase-nested pools: all three close after P5, then the P6
            # pool reuses their space
            attn_cm = tc.tile_pool(name="attnp", bufs=1)
            attnp = attn_cm.__enter__()
            seq_cm = tc.tile_pool(name="seqp", bufs=1)
            seqp = seq_cm.__enter__()
            wpin_cm = tc.tile_pool(name="wpin", bufs=1)
            wpin = wpin_cm.__enter__()

            # ---------- tiles ----------
            ident = pin.tile([128, 128], fdt)
            ssum = pin.tile([M, E], fdt)
            eadd = pin.tile([E, 1], F32)
            epsb = pin.tile([128, 2], F32)
            ohx = pin.tile([E, RPC], fdt)
            ohy = pin.tile([E, RPC], fdt)
            bh = pin.tile([128, EMB // 128], F32)
            bt = pin.tile([128, EMB // 128], F32)
            bbc = pin.tile([NCL, 1], F32)
            repm = pin.tile([E, 32, 128], fdt)
            ent = mid.tile([M, H], F32)
            if P3_FP8:
                attn = attnp.tile([64, 2, 8, NH, 128], FP8)
                ohxy2 = pin.tile([64, 2, 2 * RPC], FP8)
                av = d["attn"].ap().rearrange(
                    "p (t lc h f) -> p t lc h f", t=2, lc=8, h=NH)
                ov = d["ohxy2"].ap().rearrange("p (t n) -> p t n", t=2)
            else:
                attn = attnp.tile([M, 8, NH, 128], fdt)
                ohxy2 = pin.tile([M, 2 * RPC], fdt)
                av = d["attn"].ap().rearrange(
                    "p (lc h f) -> p lc h f", lc=8, h=NH)
                ov = d["ohxy2"].ap()
            sq = seqp.tile([128, 8, L + 1], fdt)
            wh_sb = wpin.tile([128, 16, EMB], fdt, name="wh_sb")
            wt_sb = wpin.tile([128, 16, EMB], fdt, name="wt_sb")

            # ---------- DMA priority order ----------
            nc.sync.dma_start(ent[:], d["ent"].ap())
            for t, key in [(ssum, "ssum"), (eadd, "eadd")]:
                nc.sync.dma_start(t[:], d[key].ap())
            nc.sync.dma_start(ohxy2[:], ov)
            if P3_FP8:
                for lc in range(8):
                    nc.sync.dma_start(attn[:, :, lc], av[:, :, lc])
            else:
                for lc in range(8):
                    nc.sync.dma_start(attn[:, lc], av[:, lc])
            nc.sync.dma_start(
                wh_sb[:], d["wh"].ap().rearrange("p (a b) -> p a b", a=16))
            nc.sync.dma_start(
                wt_sb[:], d["wt"].ap().rearrange("p (a b) -> p a b", a=16))
            nc.sync.dma_start(sq[:], d["seq"].ap()
                              .rearrange("p (a b) -> p a b", a=8))
            for t, key in [
                (ident, "ident"), (ohx, "ohx"), (ohy, "ohy"), (epsb, "epsb"),
                (bh, "bh"), (bt, "bt"), (bbc, "bbc"),
            ]:
                nc.sync.dma_start(t[:], d[key].ap())
            nc.scalar.dma_start(repm[:], d["repm"].ap()
                                .rearrange("p (a b) -> p a b", a=32))

            # wb stream: prefetch first 4 chunks now (scalar queue) into
            # persistent tiles; the rest rotate through the P6 pool
            wb_tiles = {}
            for k in range(4):
                wb = mid.tile([128, 32 * NCL], fdt, name=f"wbp{k}")
                nc.scalar.dma_start(
                    wb[:], d["wb"].ap()[:, k * 32 * NCL:(k + 1) * 32 * NCL])
                wb_tiles[k] = wb

            # ---------- P1: exp + segment-sum + log ----------
            psA_cm = tc.tile_pool(name="psA", bufs=1, space="PSUM")
            psA = psA_cm.__enter__()
            pexp = mid.tile([M, H], fdt, name="pexp")
            nc.scalar.activation(pexp[:], ent[:], AF.Exp)
            ps_ent = psA.tile([E, H], F32)
            for nh in range(2):
                self.mm(ps_ent[:, nh * 512:(nh + 1) * 512], ssum[:],
                        pexp[:, nh * 512:(nh + 1) * 512])
            ent_sb = mid.tile([E, H], fdt)
            nc.scalar.activation(ent_sb[:], ps_ent[:], AF.Ln, bias=eadd[:])
            psA_cm.__exit__(None, None, None)

            # entT: [h-part, hc, e]
            psT_cm = tc.tile_pool(name="psT", bufs=2, space="PSUM")
            psT = psT_cm.__enter__()
            entT = mid.tile([128, 8, E], fdt, name="entT")
            for hc in range(8):
                ps_t2 = psT.tile([128, E], fdt, tag="tp")
                self.tp(ps_t2[:], ent_sb[:, hc * 128:(hc + 1) * 128],
                        ident[0:E, 0:E])
                nc.scalar.copy(entT[:, hc, :], ps_t2[:])
            psT_cm.__exit__(None, None, None)

            # ---------- P3: C = sum_h gather_x(attn_h) * gather_y(attn_h) --
            CTmm = mid.tile([128, 8, RPC], fdt, name="CTmm")
            psP_cm = tc.tile_pool(name="psP", bufs=1, space="PSUM")
            psP = psP_cm.__enter__()
            for lc in range(8):
                prods = []
                for q in range(4):
                    psq = psP.tile([128, 4, 2, RPC], F32, tag="p3", bufs=2,
                                   name="psq")
                    for hh in range(4):
                        h = q * 4 + hh
                        if P3_FP8:
                            self.mm(psq[:, hh], attn[:, :, lc, h, :],
                                    ohxy2[:], perf_mode=DR)
                        else:
                            self.mm(psq[:, hh], attn[:, lc, h, :], ohxy2[:])
                    prod = prodp.tile([128, 4, RPC], fdt, tag=f"prod{q}",
                                      bufs=2, name=f"prod{q}")
                    route = P3_ROUTES[q]
                    if route in ("b", "g"):
                        g16 = prodp.tile([128, 4, 2, RPC], fdt, tag="g16",
                                         bufs=2, name="g16")
                        nc.scalar.copy(g16[:], psq[:])
                        eng = nc.gpsimd if route == "g" else nc.vector
                        eng.tensor_mul(prod[:], g16[:, :, 0, :],
                                       g16[:, :, 1, :])
                    else:
                        gx = prodp.tile([128, 4, RPC], fdt, tag="gx",
                                        bufs=2, name="gx")
                        nc.scalar.copy(gx[:], psq[:, :, 0, :])
                        nc.vector.tensor_mul(prod[:], gx[:], psq[:, :, 1, :])
                    prods.append(prod)
                nc.vector.tensor_add(prods[0][:], prods[0][:], prods[1][:])
                if P3_ROUTES[3] == "g":
                    nc.gpsimd.tensor_add(prods[2][:], prods[2][:],
                                         prods[3][:])
                else:
                    nc.vector.tensor_add(prods[2][:], prods[2][:],
                                         prods[3][:])
                nc.vector.tensor_add(prods[0][:], prods[0][:], prods[2][:])
                f2 = prodp.tile([128, 2, RPC], fdt, tag="fold", bufs=2,
                                name="fold")
                nc.vector.tensor_add(f2[:], prods[0][:, 0:2, :],
                                     prods[0][:, 2:4, :])
                nc.vector.tensor_add(CTmm[:, lc, :], f2[:, 0, :], f2[:, 1, :])
            psP_cm.__exit__(None, None, None)

            # ---------- EW = ent_sb @ W[0:1024] (fills the P3->P4 gap) ----
            psEW_cm = tc.tile_pool(name="psEW", bufs=2, space="PSUM")
            psEW = psEW_cm.__enter__()
            EWh = mid.tile([E, EMB], fdt, name="EWh")
            EWt = mid.tile([E, EMB], fdt, name="EWt")
            for w, ew in ((wh_sb, EWh), (wt_sb, EWt)):
                ps_ew = psEW.tile([E, EMB], F32, tag="ew")
                for hc in range(8):
                    for lo, hi in ((0, 512), (512, 768)):
                        self.mm(ps_ew[:, lo:hi], entT[:, hc, :],
                                w[:, hc, lo:hi],
                                start=(hc == 0), stop=(hc == 7))
                nc.scalar.copy(ew[:], ps_ew[:])
            psEW_cm.__exit__(None, None, None)

            # ---------- P4: rel = normalize(C) @ seq ----------
            psR_cm = tc.tile_pool(name="psR", bufs=1, space="PSUM")
            psR = psR_cm.__enter__()
            ps_rel = [psR.tile([128, L], F32, name=f"ps_rel{i}")
                      for i in range(2)]
            ps_s8 = psR.tile([128, 2, 8], F32, name="ps_s8")
            for lc in range(8):
                st, sp = lc == 0, lc == 7
                for rc in range(2):
                    lhsT = CTmm[:, lc, rc * 128:(rc + 1) * 128]
                    self.mm(ps_rel[rc][:, 0:512], lhsT, sq[:, lc, 0:512],
                            start=st, stop=sp)
                    self.mm(ps_rel[rc][:, 512:1024], lhsT, sq[:, lc, 512:1024],
                            start=st, stop=sp)
                    self.mm(ps_s8[:, rc, lc:lc + 1], lhsT,
                            sq[:, lc, 1024:1025], start=True, stop=True)
            relT = mid.tile([128, 8, RPC], fdt, name="relT")
            psT2_cm = tc.tile_pool(name="psT2", bufs=2, space="PSUM")
            psT2 = psT2_cm.__enter__()
            for rc in range(2):
                tdenom = prodp.tile([128, 1], F32, tag="tden")
                nc.vector.tensor_reduce(tdenom[:], ps_s8[:, rc, :],
                                        axis=mybir.AxisListType.X,
                                        op=mybir.AluOpType.add)
                nc.scalar.activation(tdenom[:], tdenom[:], AF.Identity,
                                     bias=epsb[:, rc:rc + 1], scale=1.0)
                frec = prodp.tile([128, 1], F32, tag="frec")
                nc.vector.reciprocal(frec[:], tdenom[:])
                rel_sc = mid.tile([128, L], fdt, tag="rel_sc", name="rel_sc")
                nc.vector.tensor_scalar_mul(rel_sc[:], ps_rel[rc][:], frec[:])
                for dc in range(8):
                    ps_t = psT2.tile([128, 128], fdt, tag="tp2")
                    self.tp(ps_t[:], rel_sc[:, dc * 128:(dc + 1) * 128],
                            ident[:])
                    nc.scalar.copy(relT[:, dc, rc * 128:(rc + 1) * 128],
                                   ps_t[:])
            psT2_cm.__exit__(None, None, None)
            psR_cm.__exit__(None, None, None)

            # ---------- P5: extractors -> hsEt/tsEt [emb, n] ----------
            psE_cm = tc.tile_pool(name="psE", bufs=4, space="PSUM")
            psE = psE_cm.__enter__()
            hsEt = mid.tile([128, 6, RPC], fdt, name="hsEt")
            tsEt = mid.tile([128, 6, RPC], fdt, name="tsEt")
            # ec-major staging: the P6 broadcast reads one contiguous 16KB
            # block per partition
            hsd = dramp.tile([6, 128, RPC], fdt, name="hsd")
            tsd = dramp.tile([6, 128, RPC], fdt, name="tsd")
            for ec in range(6):
                for (w, bvec, ew, oh, dst, dstd) in (
                    (wh_sb, bh, EWh, ohx, hsEt, hsd),
                    (wt_sb, bt, EWt, ohy, tsEt, tsd),
                ):
                    ps_e = psE.tile([128, RPC], F32, tag="pe", name="ps_e")
                    self.mm(ps_e[:], ew[:, ec * 128:(ec + 1) * 128], oh[:],
                            start=True, stop=False)
                    for kc in range(8, 16):
                        self.mm(ps_e[:], w[:, kc, ec * 128:(ec + 1) * 128],
                                relT[:, kc % 8, :],
                                start=False, stop=(kc == 15))
                    nc.scalar.activation(dst[:, ec, :], ps_e[:], AF.Tanh,
                                         bias=bvec[:, ec:ec + 1])
                    nc.scalar.dma_start(dstd[ec], dst[:, ec, :])
            psE_cm.__exit__(None, None, None)
            wpin_cm.__exit__(None, None, None)
            seq_cm.__exit__(None, None, None)
            attn_cm.__exit__(None, None, None)

            # ---------- P6: block bilinear + classifier ----------
            with (
                tc.tile_pool(name="blph", bufs=1) as blph,
                tc.tile_pool(name="ps_lt", bufs=1, space="PSUM") as ps_lt,
                tc.tile_pool(name="psRep", bufs=2, space="PSUM") as psRep,
            ):
                pslt = ps_lt.tile([NCL, RPC], F32)

                def issue_b2t(k, tag="b2t", bufs=3):
                    kk = 64 * (k % 2)
                    ec = k // 2
                    b2t = blph.tile([128, RPC], fdt, tag=tag, bufs=bufs,
                                    name=tag)
                    for h0 in (0, 1):
                        nc.sync.dma_start(b2t[64 * h0:64 * (h0 + 1)],
                                          tsd[ec, kk:kk + 64, :])
                    return b2t

                def issue_b1rep(k, bufs=3, tag="b1rep"):
                    kk = 64 * (k % 2)
                    ec = k // 2
                    b1rep = blph.tile([128, 32, RPC], fdt, tag=tag, bufs=bufs,
                                      name=tag)
                    for h0 in (0, 1):
                        src = hsd[ec, kk + 32 * h0:kk + 32 * (h0 + 1), :] \
                            .unsqueeze(0).broadcast_to([64, 32, RPC])
                        nc.sync.dma_start(b1rep[64 * h0:64 * (h0 + 1)], src)
                    return b1rep

                cg = 0
                for k in range(K12):
                    kk = 64 * (k % 2)
                    ec = k // 2
                    route = P6_ROUTES[k]
                    if k in wb_tiles:
                        wb = wb_tiles[k]
                    else:
                        wb = blph.tile([128, 32 * NCL], fdt, tag="wb",
                                       bufs=3, name="wb")
                        nc.scalar.dma_start(
                            wb[:],
                            d["wb"].ap()[:, k * 32 * NCL:(k + 1) * 32 * NCL])
                    b2t = issue_b2t(k)
                    blT = blph.tile([128, 32, RPC], fdt, tag="blT",
                                    bufs=2, name="blT")
                    if route in ("pd", "pa"):
                        assert kk == 0, "PE replication route needs even k"
                        hsE64 = hsEt[kk:kk + 64, ec, :]
                        b2b = b2t[:].unsqueeze(1).broadcast_to([128, 4, RPC])
                        for cq in range(8):
                            psq6 = psRep.tile([128, 4, RPC], F32, tag="rep",
                                              bufs=2, name="psq6")
                            for i4 in range(4):
                                self.mm(psq6[:, i4, :],
                                        repm[:, cq * 4 + i4, :], hsE64)
                            if route == "pa":
                                b1c = blph.tile([128, 4, RPC], fdt, tag="b1c",
                                                bufs=3, name="b1c")
                                nc.scalar.copy(b1c[:], psq6[:])
                                nc.vector.tensor_mul(
                                    blT[:, cq * 4:(cq + 1) * 4, :],
                                    b1c[:], b2b)
                            else:
                                nc.vector.tensor_mul(
                                    blT[:, cq * 4:(cq + 1) * 4, :],
                                    psq6[:], b2b)
                    else:
                        b1rep = issue_b1rep(k)
                        b2b = b2t[:].unsqueeze(1).broadcast_to([128, 8, RPC])
                        for g in range(4):
                            nc.vector.tensor_mul(
                                blT[:, g * 8:(g + 1) * 8, :],
                                b1rep[:, g * 8:(g + 1) * 8, :], b2b)
                    for c in range(32):
                        self.mm(pslt[:], wb[:, c * NCL:(c + 1) * NCL],
                                blT[:, c, :],
                                start=(cg == 0), stop=(cg == 383))
                        cg += 1

                out_sb = mid.tile([NCL, RPC], F32)
                nc.scalar.activation(out_sb[:], pslt[:], AF.Identity,
                                     bias=bbc[:])
                nc.sync.dma_start(d["lt"].ap(), out_sb[:])


_PROGRAM_CACHE = {}


def _get_program():
    if _FULL_KEY not in _PROGRAM_CACHE:
        _PROGRAM_CACHE[_FULL_KEY] = _Builder()
    return _PROGRAM_CACHE[_FULL_KEY]


def _host_inputs(seq_lhs, ent_lhs, ent_to_seq_attn, entity_id_labels, hts,
                 Wh, bh, Wt, bt, Wb, bb):
    """Build the 8 per-core input maps (all host-side numpy)."""
    fdt = _np_fdt()
    seq_lhs = np.asarray(seq_lhs, np.float32)
    ent_lhs = np.asarray(ent_lhs, np.float32)
    ent_to_seq_attn = np.asarray(ent_to_seq_attn, np.float32)
    entity_id_labels = np.asarray(entity_id_labels)
    hts = np.asarray(hts)
    Wh = np.asarray(Wh, np.float32)
    Wt = np.asarray(Wt, np.float32)
    Wb = np.asarray(Wb, np.float32)
    bh = np.asarray(bh, np.float32)
    bt = np.asarray(bt, np.float32)
    bb = np.asarray(bb, np.float32)

    # device chunk (k, c) row p maps to Wb row k*4096 + i*64 + j with
    # i = c + 32*(p//64), j = p%64
    p_ = np.arange(128)
    c_ = np.arange(32)
    k_ = np.arange(K12)
    rows = (k_[:, None, None] * 4096
            + (c_[None, :, None] + 32 * (p_[None, None, :] // 64)) * 64
            + (p_[None, None, :] % 64))  # [k, c, p]
    wb_r = np.ascontiguousarray(
        Wb[rows.reshape(-1), :].reshape(K12 * 32, 128, NCL)
        .transpose(1, 0, 2).reshape(128, 384 * NCL)
    ).astype(fdt)
    wh_c = np.ascontiguousarray(
        Wh.reshape(16, 128, EMB).transpose(1, 0, 2).reshape(128, 16 * EMB)
    ).astype(fdt)
    wt_c = np.ascontiguousarray(
        Wt.reshape(16, 128, EMB).transpose(1, 0, 2).reshape(128, 16 * EMB)
    ).astype(fdt)
    bh_c = np.ascontiguousarray(bh.reshape(EMB // 128, 128).T)
    bt_c = np.ascontiguousarray(bt.reshape(EMB // 128, 128).T)
    bb_c = np.ascontiguousarray(bb.reshape(NCL, 1))
    ident = np.eye(128, dtype=np.float32).astype(fdt)
    # repm[r, c, p] = 1 iff r == c + 32*(p//64)
    repm_h = np.zeros((E, 32, 128), np.float32)
    for c in range(32):
        repm_h[c, c, 0:64] = 1.0
        repm_h[c + 32, c, 64:128] = 1.0
    repm_h = repm_h.reshape(E, 32 * 128).astype(fdt)

    in_maps = []
    for c in range(NCORES):
        doc, half = divmod(c, 2)
        sl = slice(half * RPC, (half + 1) * RPC)
        labels = entity_id_labels[doc].astype(np.int64)
        cnt = np.bincount(labels, minlength=E).astype(np.float32)
        S = np.zeros((M, E), np.float32)
        S[np.arange(M), labels] = 1.0
        eadd = (cnt == 0).astype(np.float32).reshape(E, 1)
        hi = hts[doc, sl, 0].astype(np.int64)
        ti = hts[doc, sl, 1].astype(np.int64)
        ohx = np.zeros((E, RPC), np.float32)
        ohx[hi, np.arange(RPC)] = 1.0
        ohy = np.zeros((E, RPC), np.float32)
        ohy[ti, np.arange(RPC)] = 1.0
        # exact-scaled gather: x columns are 0/1, y columns are 0 or 1/64
        # (both exact in fp8/fp16); the per-pair 1/(cntx*cnty) mean factor
        # cancels in the rel normalization, with the 1e-5 eps rescaled by
        # epsb = 16e-5 * cntx * cnty / 64 per pair
        ohxs = (S @ ohx)            # [M, RPC] in {0, 1}
        ohys = (S @ ohy) * (1.0 / 64.0)
        cnt1 = np.maximum(cnt, 1.0)
        scl = cnt1[hi] * cnt1[ti] / 64.0    # [RPC]
        epsb = np.ascontiguousarray(
            (16e-5 * scl).reshape(2, 128).T.astype(np.float32))  # [128, 2]
        ohxy2 = np.concatenate([ohxs, ohys], axis=1)  # [M, 512]
        a = (ent_to_seq_attn[doc].transpose(1, 0, 2)  # [M, NH, L]
             .reshape(M, NH, 8, 128).transpose(0, 2, 1, 3))  # [M, 8, NH, 128]
        if P3_FP8:
            f8 = _np_fp8()
            # m = t*64 + k  ->  [k(64), t(2), lc, h, f]
            attn_c = np.ascontiguousarray(
                a.reshape(2, 64, 8, NH, 128).transpose(1, 0, 2, 3, 4)
                .reshape(64, 2 * 8 * NH * 128)).astype(f8)
            ohxy2_c = np.ascontiguousarray(
                ohxy2.reshape(2, 64, 2 * RPC).transpose(1, 0, 2)
                .reshape(64, 2 * 2 * RPC)).astype(f8)
        else:
            attn_c = np.ascontiguousarray(
                a.reshape(M, 8 * NH * 128)).astype(fdt)
            ohxy2_c = ohxy2.astype(fdt)
        seq_r = seq_lhs[doc].reshape(8, 128, L).transpose(1, 0, 2)
        seq_aug = np.concatenate(
            [seq_r, np.ones((128, 8, 1), np.float32)], axis=2
        )
        in_maps.append({
            "ent": np.ascontiguousarray(ent_lhs[doc]),
            "attn": attn_c,
            "seq": np.ascontiguousarray(
                seq_aug.reshape(128, 8 * (L + 1))).astype(fdt),
            "ssum": S.astype(fdt),
            "ohxy2": ohxy2_c,
            "epsb": epsb,
            "eadd": eadd,
            "ohx": ohx.astype(fdt),
            "ohy": ohy.astype(fdt),
            "wh": wh_c, "wt": wt_c, "bh": bh_c, "bt": bt_c,
            "wb": wb_r, "bbc": bb_c, "ident": ident, "repm": repm_h,
        })
    return in_maps


_LAST_RESULTS = {}


def kernel(**inputs) -> np.ndarray:
    prog = _get_program()
    in_maps = _host_inputs(**inputs)
    trace = os.environ.get("DOCRED_TRACE", "0") == "1"
    res = run_bass_kernel_spmd(
        prog.nc, in_maps, core_ids=list(range(NCORES)), trace=trace,
    )
    _LAST_RESULTS["res"] = res
    out = np.empty((B * R, NCL), np.float32)
    for c in range(NCORES):
        doc, half = divmod(c, 2)
        lt = res.results[c]["lt"]  # [NCL, RPC]
        out[doc * R + half * RPC: doc * R + (half + 1) * RPC, :] = lt.T
    return out


# revision 34
# speedup vs baseline: 1.0858x; 1.0858x over previous
"""Trainium2 Bass kernel for the DocRED-style segment_reduce model.

Sharding: 8 cores, data-parallel: core c -> (doc = c//2, pair-half = c%2).
Each core independently computes logits for its 256 pairs. No collectives.
All segment reductions / gathers are lowered to one-hot matmuls whose
one-hot matrices are built on the host from the integer inputs and passed
as per-core input tensors (the SPMD program itself is index-agnostic).

Key facts driving the design (measured on HW):
  - the PE throttles to ~50% utilization under sustained load
    (throttle_activity_1_avg_util_limit=0.5), so total PE cycles are the
    main budget; short bursts run at 2.4 GHz, sustained streams ~1.2 GHz
  - DVE/GPSIMD can read at most one (DVE) / zero (GPSIMD) PSUM operands,
    so every PE result used by a product must be drained by ACT first
  - the attention-gather one-hots are EXACT 0/1 (or 1/64) matrices: the
    segment-mean scale cancels in the rel normalization, so the gather
    matmuls can run in fp8e4m3 DoubleRow (2x PE throughput) with only
    the attention values themselves quantized (DOCRED_P3_FP8=1)
"""

import os

import numpy as np

import concourse.bacc as bacc
import concourse.bass as bass
import concourse.mybir as mybir
import concourse.tile as tile
from concourse.bass_utils import run_bass_kernel_spmd

B, M, H = 4, 128, 1024
NH, L = 16, 1024
E, R = 64, 512
EMB, BS, NCL = 768, 64, 97
K12 = EMB // BS  # 12 blocks
NCORES = 8
RPC = R // 2  # pairs per core

F32 = mybir.dt.float32
F16 = mybir.dt.float16
BF16 = mybir.dt.bfloat16
FP8 = mybir.dt.float8e4

MM_MODE = os.environ.get("DOCRED_MM_MODE", "f16")
# P3 gather matmuls in fp8e4m3 DoubleRow (2x PE throughput)
P3_FP8 = os.environ.get("DOCRED_P3_FP8", "0") == "1"
# P3 per-pack product route ("a": ACT copies x-half, DVE mults SBUF x PSUM;
# "b": ACT copies both, DVE fp16; "g": ACT copies both, GpSimd mults)
P3_ROUTES = os.environ.get("DOCRED_P3_ROUTES", "a,a,a,g").split(",")
# P6 k-block routes ("pa": PE one-hot replication + ACT copy (even k only);
# "pd": PE replication, DVE mults from PSUM; "d": DMA broadcast replication)
P6_ROUTES = os.environ.get(
    "DOCRED_P6_ROUTES", "pa,d,d,d,pa,d,d,d,d,d,d,d").split(",")

_FULL_KEY = (MM_MODE, P3_FP8, tuple(P3_ROUTES), tuple(P6_ROUTES))


def _fdt():
    return BF16 if MM_MODE == "bf16" else F16


def _np_fdt():
    import ml_dtypes

    return np.dtype(ml_dtypes.bfloat16) if MM_MODE == "bf16" else np.float16


def _np_fp8():
    import ml_dtypes

    return np.dtype(ml_dtypes.float8_e4m3)


class _Builder:
    def __init__(self):
        self.fdt = _fdt()
        nc = bacc.Bacc("TRN2", target_bir_lowering=False, debug=False)
        self.nc = nc
        fdt = self.fdt
        d = {}
        d["ent"] = nc.dram_tensor("ent", [M, H], F32, kind="ExternalInput")
        if P3_FP8:
            # [k(64), t(2), lc, h, 128] fp8 (m = t*64 + k)
            d["attn"] = nc.dram_tensor("attn", [64, 2 * 8 * NH * 128], FP8,
                                       kind="ExternalInput")
            d["ohxy2"] = nc.dram_tensor("ohxy2", [64, 2 * 2 * RPC], FP8,
                                        kind="ExternalInput")
        else:
            d["attn"] = nc.dram_tensor("attn", [M, 8 * NH * 128], fdt,
                                       kind="ExternalInput")
            d["ohxy2"] = nc.dram_tensor("ohxy2", [M, 2 * RPC], fdt,
                                        kind="ExternalInput")
        d["epsb"] = nc.dram_tensor("epsb", [128, 2], F32, kind="ExternalInput")
        d["seq"] = nc.dram_tensor("seq", [128, 8 * (L + 1)], fdt,
                                  kind="ExternalInput")
        d["ssum"] = nc.dram_tensor("ssum", [M, E], fdt, kind="ExternalInput")
        d["eadd"] = nc.dram_tensor("eadd", [E, 1], F32, kind="ExternalInput")
        d["ohx"] = nc.dram_tensor("ohx", [E, RPC], fdt, kind="ExternalInput")
        d["ohy"] = nc.dram_tensor("ohy", [E, RPC], fdt, kind="ExternalInput")
        d["wh"] = nc.dram_tensor("wh", [128, 16 * EMB], fdt, kind="ExternalInput")
        d["wt"] = nc.dram_tensor("wt", [128, 16 * EMB], fdt, kind="ExternalInput")
        d["bh"] = nc.dram_tensor("bh", [128, EMB // 128], F32, kind="ExternalInput")
        d["bt"] = nc.dram_tensor("bt", [128, EMB // 128], F32, kind="ExternalInput")
        d["wb"] = nc.dram_tensor("wb", [128, 384 * NCL], fdt, kind="ExternalInput")
        d["bbc"] = nc.dram_tensor("bbc", [NCL, 1], F32, kind="ExternalInput")
        d["ident"] = nc.dram_tensor("ident", [128, 128], fdt, kind="ExternalInput")
        d["repm"] = nc.dram_tensor("repm", [E, 32 * 128], fdt, kind="ExternalInput")
        d["lt"] = nc.dram_tensor("lt", [NCL, RPC], F32, kind="ExternalOutput")
        self.d = d
        with tile.TileContext(nc) as tc:
            self.build(tc)
        nc.compile()

    def mm(self, out, lhsT, rhs, **kw):
        return self.nc.tensor.matmul(out, lhsT, rhs, **kw)

    def tp(self, out, in_, ident, **kw):
        return self.nc.tensor.matmul(out, in_, ident, is_transpose=True, **kw)

    def build(self, tc):
        nc = self.nc
        d = self.d
        fdt = self.fdt
        AF = mybir.ActivationFunctionType
        DR = mybir.MatmulPerfMode.DoubleRow

        with (
            tc.tile_pool(name="pin", bufs=1) as pin,
            tc.tile_pool(name="mid", bufs=1) as mid,
            tc.tile_pool(name="prodp", bufs=2) as prodp,
            tc.tile_pool(name="dramp", bufs=1, space="DRAM") as dramp,
        ):
            # phase-nested pools: all three close after P5, then the P6
            # pool reuses their space
            attn_cm = tc.tile_pool(name="attnp", bufs=1)
            attnp = attn_cm.__enter__()
            seq_cm = tc.tile_pool(name="seqp", bufs=1)
            seqp = seq_cm.__enter__()
            wpin_cm = tc.tile_pool(name="wpin", bufs=1)
            wpin = wpin_cm.__enter__()

            # ---------- tiles ----------
            ident = pin.tile([128, 128], fdt)
            ssum = pin.tile([M, E], fdt)
            eadd = pin.tile([E, 1], F32)
            epsb = pin.tile([128, 2], F32)
            ohx = pin.tile([E, RPC], fdt)
            ohy = pin.tile([E, RPC], fdt)
            bh = pin.tile([128, EMB // 128], F32)
            bt = pin.tile([128, EMB // 128], F32)
            bbc = pin.tile([NCL, 1], F32)
            repm = pin.tile([E, 32, 128], fdt)
            ent = mid.tile([M, H], F32)
            if P3_FP8:
                attn = attnp.tile([64, 2, 8, NH, 128], FP8)
                ohxy2 = pin.tile([64, 2, 2 * RPC], FP8)
                av = d["attn"].ap().rearrange(
                    "p (t lc h f) -> p t lc h f", t=2, lc=8, h=NH)
                ov = d["ohxy2"].ap().rearrange("p (t n) -> p t n", t=2)
            else:
                attn = attnp.tile([M, 8, NH, 128], fdt)
                ohxy2 = pin.tile([M, 2 * RPC], fdt)
                av = d["attn"].ap().rearrange(
                    "p (lc h f) -> p lc h f", lc=8, h=NH)
                ov = d["ohxy2"].ap()
            sq = seqp.tile([128, 8, L + 1], fdt)
            wh_sb = wpin.tile([128, 16, EMB], fdt, name="wh_sb")
            wt_sb = wpin.tile([128, 16, EMB], fdt, name="wt_sb")

            # ---------- DMA priority order ----------
            nc.sync.dma_start(ent[:], d["ent"].ap())
            for t, key in [(ssum, "ssum"), (eadd, "eadd")]:
                nc.sync.dma_start(t[:], d[key].ap())
            nc.sync.dma_start(ohxy2[:], ov)
            for t, key in [
                (ident, "ident"), (ohx, "ohx"), (ohy, "ohy"), (epsb, "epsb"),
                (bh, "bh"), (bt, "bt"), (bbc, "bbc"),
            ]:
                nc.sync.dma_start(t[:], d[key].ap())
            if P3_FP8:
                for lc in range(8):
                    nc.sync.dma_start(attn[:, :, lc], av[:, :, lc])
            else:
                for lc in range(8):
                    nc.sync.dma_start(attn[:, lc], av[:, lc])
            nc.sync.dma_start(
                wh_sb[:], d["wh"].ap().rearrange("p (a b) -> p a b", a=16))
            nc.sync.dma_start(
                wt_sb[:], d["wt"].ap().rearrange("p (a b) -> p a b", a=16))
            nc.sync.dma_start(sq[:], d["seq"].ap()
                              .rearrange("p (a b) -> p a b", a=8))
            nc.scalar.dma_start(repm[:], d["repm"].ap()
                                .rearrange("p (a b) -> p a b", a=32))

            # wb stream: prefetch first 4 chunks now (scalar queue) into
            # persistent tiles; the rest rotate through the P6 pool
            wb_tiles = {}
            for k in range(4):
                wb = mid.tile([128, 32 * NCL], fdt, name=f"wbp{k}")
                nc.scalar.dma_start(
                    wb[:], d["wb"].ap()[:, k * 32 * NCL:(k + 1) * 32 * NCL])
                wb_tiles[k] = wb

            # ---------- P1: exp + segment-sum + log ----------
            psA_cm = tc.tile_pool(name="psA", bufs=1, space="PSUM")
            psA = psA_cm.__enter__()
            pexp = mid.tile([M, H], fdt, name="pexp")
            nc.scalar.activation(pexp[:], ent[:], AF.Exp)
            ps_ent = psA.tile([E, H], F32)
            for nh in range(2):
                self.mm(ps_ent[:, nh * 512:(nh + 1) * 512], ssum[:],
                        pexp[:, nh * 512:(nh + 1) * 512])
            ent_sb = mid.tile([E, H], fdt)
            nc.scalar.activation(ent_sb[:], ps_ent[:], AF.Ln, bias=eadd[:])
            psA_cm.__exit__(None, None, None)

            # ---------- P3: C = sum_h gather_x(attn_h) * gather_y(attn_h) --
            CTmm = mid.tile([128, 8, RPC], fdt, name="CTmm")
            psP_cm = tc.tile_pool(name="psP", bufs=1, space="PSUM")
            psP = psP_cm.__enter__()
            for lc in range(8):
                prods = []
                for q in range(4):
                    psq = psP.tile([128, 4, 2, RPC], F32, tag="p3", bufs=2,
                                   name="psq")
                    for hh in range(4):
                        h = q * 4 + hh
                        if P3_FP8:
                            self.mm(psq[:, hh], attn[:, :, lc, h, :],
                                    ohxy2[:], perf_mode=DR)
                        else:
                            self.mm(psq[:, hh], attn[:, lc, h, :], ohxy2[:])
                    prod = prodp.tile([128, 4, RPC], fdt, tag=f"prod{q}",
                                      bufs=2, name=f"prod{q}")
                    route = P3_ROUTES[q]
                    if route in ("b", "g"):
                        g16 = prodp.tile([128, 4, 2, RPC], fdt, tag="g16",
                                         bufs=2, name="g16")
                        nc.scalar.copy(g16[:], psq[:])
                        eng = nc.gpsimd if route == "g" else nc.vector
                        eng.tensor_mul(prod[:], g16[:, :, 0, :],
                                       g16[:, :, 1, :])
                    else:
                        gx = prodp.tile([128, 4, RPC], fdt, tag="gx",
                                        bufs=2, name="gx")
                        nc.scalar.copy(gx[:], psq[:, :, 0, :])
                        nc.vector.tensor_mul(prod[:], gx[:], psq[:, :, 1, :])
                    prods.append(prod)
                nc.vector.tensor_add(prods[0][:], prods[0][:], prods[1][:])
                if P3_ROUTES[3] == "g":
                    nc.gpsimd.tensor_add(prods[2][:], prods[2][:],
                                         prods[3][:])
                else:
                    nc.vector.tensor_add(prods[2][:], prods[2][:],
                                         prods[3][:])
                nc.vector.tensor_add(prods[0][:], prods[0][:], prods[2][:])
                f2 = prodp.tile([128, 2, RPC], fdt, tag="fold", bufs=2,
                                name="fold")
                nc.vector.tensor_add(f2[:], prods[0][:, 0:2, :],
                                     prods[0][:, 2:4, :])
                nc.vector.tensor_add(CTmm[:, lc, :], f2[:, 0, :], f2[:, 1, :])
            psP_cm.__exit__(None, None, None)

            # entT: [h-part, hc, e] (issued after P3 so the PE queue never
            # stalls on the ident DMA)
            psT_cm = tc.tile_pool(name="psT", bufs=2, space="PSUM")
            psT = psT_cm.__enter__()
            entT = mid.tile([128, 8, E], fdt, name="entT")
            for hc in range(8):
                ps_t2 = psT.tile([128, E], fdt, tag="tp")
                self.tp(ps_t2[:], ent_sb[:, hc * 128:(hc + 1) * 128],
                        ident[0:E, 0:E])
                nc.scalar.copy(entT[:, hc, :], ps_t2[:])
            psT_cm.__exit__(None, None, None)

            # ---------- EW = ent_sb @ W[0:1024] (fills the P3->P4 gap) ----
            psEW_cm = tc.tile_pool(name="psEW", bufs=2, space="PSUM")
            psEW = psEW_cm.__enter__()
            EWh = mid.tile([E, EMB], fdt, name="EWh")
            EWt = mid.tile([E, EMB], fdt, name="EWt")
            for w, ew in ((wh_sb, EWh), (wt_sb, EWt)):
                ps_ew = psEW.tile([E, EMB], F32, tag="ew")
                for hc in range(8):
                    for lo, hi in ((0, 512), (512, 768)):
                        self.mm(ps_ew[:, lo:hi], entT[:, hc, :],
                                w[:, hc, lo:hi],
                                start=(hc == 0), stop=(hc == 7))
                nc.scalar.copy(ew[:], ps_ew[:])
            psEW_cm.__exit__(None, None, None)

            # ---------- P4: rel = normalize(C) @ seq ----------
            psR_cm = tc.tile_pool(name="psR", bufs=1, space="PSUM")
            psR = psR_cm.__enter__()
            ps_rel = [psR.tile([128, L], F32, name=f"ps_rel{i}")
                      for i in range(2)]
            ps_s8 = psR.tile([128, 2, 8], F32, name="ps_s8")
            for lc in range(8):
                st, sp = lc == 0, lc == 7
                for rc in range(2):
                    lhsT = CTmm[:, lc, rc * 128:(rc + 1) * 128]
                    self.mm(ps_rel[rc][:, 0:512], lhsT, sq[:, lc, 0:512],
                            start=st, stop=sp)
                    self.mm(ps_rel[rc][:, 512:1024], lhsT, sq[:, lc, 512:1024],
                            start=st, stop=sp)
                    self.mm(ps_s8[:, rc, lc:lc + 1], lhsT,
                            sq[:, lc, 1024:1025], start=True, stop=True)
            relT = mid.tile([128, 8, RPC], fdt, name="relT")
            psT2_cm = tc.tile_pool(name="psT2", bufs=2, space="PSUM")
            psT2 = psT2_cm.__enter__()
            for rc in range(2):
                tdenom = prodp.tile([128, 1], F32, tag="tden")
                nc.vector.tensor_reduce(tdenom[:], ps_s8[:, rc, :],
                                        axis=mybir.AxisListType.X,
                                        op=mybir.AluOpType.add)
                nc.scalar.activation(tdenom[:], tdenom[:], AF.Identity,
                                     bias=epsb[:, rc:rc + 1], scale=1.0)
                frec = prodp.tile([128, 1], F32, tag="frec")
                nc.vector.reciprocal(frec[:], tdenom[:])
                rel_sc = mid.tile([128, L], fdt, tag="rel_sc", name="rel_sc")
                nc.vector.tensor_scalar_mul(rel_sc[:], ps_rel[rc][:], frec[:])
                for dc in range(8):
                    ps_t = psT2.tile([128, 128], fdt, tag="tp2")
                    self.tp(ps_t[:], rel_sc[:, dc * 128:(dc + 1) * 128],
                            ident[:])
                    nc.scalar.copy(relT[:, dc, rc * 128:(rc + 1) * 128],
                                   ps_t[:])
            psT2_cm.__exit__(None, None, None)
            psR_cm.__exit__(None, None, None)

            # ---------- P5: extractors -> hsEt/tsEt [emb, n] ----------
            psE_cm = tc.tile_pool(name="psE", bufs=4, space="PSUM")
            psE = psE_cm.__enter__()
            hsEt = mid.tile([128, 6, RPC], fdt, name="hsEt")
            tsEt = mid.tile([128, 6, RPC], fdt, name="tsEt")
            # ec-major staging: the P6 broadcast reads one contiguous 16KB
            # block per partition
            hsd = dramp.tile([6, 128, RPC], fdt, name="hsd")
            tsd = dramp.tile([6, 128, RPC], fdt, name="tsd")
            for ec in range(6):
                for (w, bvec, ew, oh, dst, dstd) in (
                    (wh_sb, bh, EWh, ohx, hsEt, hsd),
                    (wt_sb, bt, EWt, ohy, tsEt, tsd),
                ):
                    ps_e = psE.tile([128, RPC], F32, tag="pe", name="ps_e")
                    self.mm(ps_e[:], ew[:, ec * 128:(ec + 1) * 128], oh[:],
                            start=True, stop=False)
                    for kc in range(8, 16):
                        self.mm(ps_e[:], w[:, kc, ec * 128:(ec + 1) * 128],
                                relT[:, kc % 8, :],
                                start=False, stop=(kc == 15))
                    nc.scalar.activation(dst[:, ec, :], ps_e[:], AF.Tanh,
                                         bias=bvec[:, ec:ec + 1])
                    nc.scalar.dma_start(dstd[ec], dst[:, ec, :])
            psE_cm.__exit__(None, None, None)
            wpin_cm.__exit__(None, None, None)
            seq_cm.__exit__(None, None, None)
            attn_cm.__exit__(None, None, None)

            # ---------- P6: block bilinear + classifier ----------
            with (
                tc.tile_pool(name="blph", bufs=1) as blph,
                tc.tile_pool(name="ps_lt", bufs=1, space="PSUM") as ps_lt,
                tc.tile_pool(name="psRep", bufs=2, space="PSUM") as psRep,
            ):
                pslt = ps_lt.tile([NCL, RPC], F32)

                def issue_b2t(k, tag="b2t", bufs=3):
                    kk = 64 * (k % 2)
                    ec = k // 2
                    b2t = blph.tile([128, RPC], fdt, tag=tag, bufs=bufs,
                                    name=tag)
                    for h0 in (0, 1):
                        nc.sync.dma_start(b2t[64 * h0:64 * (h0 + 1)],
                                          tsd[ec, kk:kk + 64, :])
                    return b2t

                def issue_b1rep(k, bufs=3, tag="b1rep"):
                    kk = 64 * (k % 2)
                    ec = k // 2
                    b1rep = blph.tile([128, 32, RPC], fdt, tag=tag, bufs=bufs,
                                      name=tag)
                    for h0 in (0, 1):
                        src = hsd[ec, kk + 32 * h0:kk + 32 * (h0 + 1), :] \
                            .unsqueeze(0).broadcast_to([64, 32, RPC])
                        nc.sync.dma_start(b1rep[64 * h0:64 * (h0 + 1)], src)
                    return b1rep

                cg = 0
                for k in range(K12):
                    kk = 64 * (k % 2)
                    ec = k // 2
                    route = P6_ROUTES[k]
                    if k in wb_tiles:
                        wb = wb_tiles[k]
                    else:
                        wb = blph.tile([128, 32 * NCL], fdt, tag="wb",
                                       bufs=3, name="wb")
                        nc.scalar.dma_start(
                            wb[:],
                            d["wb"].ap()[:, k * 32 * NCL:(k + 1) * 32 * NCL])
                    b2t = issue_b2t(k)
                    blT = blph.tile([128, 32, RPC], fdt, tag="blT",
                                    bufs=2, name="blT")
                    if route in ("pd", "pa"):
                        assert kk == 0, "PE replication route needs even k"
                        hsE64 = hsEt[kk:kk + 64, ec, :]
                        b2b = b2t[:].unsqueeze(1).broadcast_to([128, 4, RPC])
                        for cq in range(8):
                            psq6 = psRep.tile([128, 4, RPC], F32, tag="rep",
                                              bufs=2, name="psq6")
                            for i4 in range(4):
                                self.mm(psq6[:, i4, :],
                                        repm[:, cq * 4 + i4, :], hsE64)
                            if route == "pa":
                                b1c = blph.tile([128, 4, RPC], fdt, tag="b1c",
                                                bufs=3, name="b1c")
                                nc.scalar.copy(b1c[:], psq6[:])
                                nc.vector.tensor_mul(
                                    blT[:, cq * 4:(cq + 1) * 4, :],
                                    b1c[:], b2b)
                            else:
                                nc.vector.tensor_mul(
                                    blT[:, cq * 4:(cq + 1) * 4, :],
                                    psq6[:], b2b)
                    else:
                        b1rep = issue_b1rep(k)
                        b2b = b2t[:].unsqueeze(1).broadcast_to([128, 8, RPC])
                        for g in range(4):
                            nc.vector.tensor_mul(
                                blT[:, g * 8:(g + 1) * 8, :],
                                b1rep[:, g * 8:(g + 1) * 8, :], b2b)
                    for c in range(32):
                        self.mm(pslt[:], wb[:, c * NCL:(c + 1) * NCL],
                                blT[:, c, :],
                                start=(cg == 0), stop=(cg == 383))
                        cg += 1

                out_sb = mid.tile([NCL, RPC], F32)
                nc.scalar.activation(out_sb[:], pslt[:], AF.Identity,
                                     bias=bbc[:])
                nc.sync.dma_start(d["lt"].ap(), out_sb[:])


_PROGRAM_CACHE = {}


def _get_program():
    if _FULL_KEY not in _PROGRAM_CACHE:
        _PROGRAM_CACHE[_FULL_KEY] = _Builder()
    return _PROGRAM_CACHE[_FULL_KEY]


def _host_inputs(seq_lhs, ent_lhs, ent_to_seq_attn, entity_id_labels, hts,
                 Wh, bh, Wt, bt, Wb, bb):
    """Build the 8 per-core input maps (all host-side numpy)."""
    fdt = _np_fdt()
    seq_lhs = np.asarray(seq_lhs, np.float32)
    ent_lhs = np.asarray(ent_lhs, np.float32)
    ent_to_seq_attn = np.asarray(ent_to_seq_attn, np.float32)
    entity_id_labels = np.asarray(entity_id_labels)
    hts = np.asarray(hts)
    Wh = np.asarray(Wh, np.float32)
    Wt = np.asarray(Wt, np.float32)
    Wb = np.asarray(Wb, np.float32)
    bh = np.asarray(bh, np.float32)
    bt = np.asarray(bt, np.float32)
    bb = np.asarray(bb, np.float32)

    # device chunk (k, c) row p maps to Wb row k*4096 + i*64 + j with
    # i = c + 32*(p//64), j = p%64
    p_ = np.arange(128)
    c_ = np.arange(32)
    k_ = np.arange(K12)
    rows = (k_[:, None, None] * 4096
            + (c_[None, :, None] + 32 * (p_[None, None, :] // 64)) * 64
            + (p_[None, None, :] % 64))  # [k, c, p]
    wb_r = np.ascontiguousarray(
        Wb[rows.reshape(-1), :].reshape(K12 * 32, 128, NCL)
        .transpose(1, 0, 2).reshape(128, 384 * NCL)
    ).astype(fdt)
    wh_c = np.ascontiguousarray(
        Wh.reshape(16, 128, EMB).transpose(1, 0, 2).reshape(128, 16 * EMB)
    ).astype(fdt)
    wt_c = np.ascontiguousarray(
        Wt.reshape(16, 128, EMB).transpose(1, 0, 2).reshape(128, 16 * EMB)
    ).astype(fdt)
    bh_c = np.ascontiguousarray(bh.reshape(EMB // 128, 128).T)
    bt_c = np.ascontiguousarray(bt.reshape(EMB // 128, 128).T)
    bb_c = np.ascontiguousarray(bb.reshape(NCL, 1))
    ident = np.eye(128, dtype=np.float32).astype(fdt)
    # repm[r, c, p] = 1 iff r == c + 32*(p//64)
    repm_h = np.zeros((E, 32, 128), np.float32)
    for c in range(32):
        repm_h[c, c, 0:64] = 1.0
        repm_h[c + 32, c, 64:128] = 1.0
    repm_h = repm_h.reshape(E, 32 * 128).astype(fdt)

    in_maps = []
    for c in range(NCORES):
        doc, half = divmod(c, 2)
        sl = slice(half * RPC, (half + 1) * RPC)
        labels = entity_id_labels[doc].astype(np.int64)
        cnt = np.bincount(labels, minlength=E).astype(np.float32)
        S = np.zeros((M, E), np.float32)
        S[np.arange(M), labels] = 1.0
        eadd = (cnt == 0).astype(np.float32).reshape(E, 1)
        hi = hts[doc, sl, 0].astype(np.int64)
        ti = hts[doc, sl, 1].astype(np.int64)
        ohx = np.zeros((E, RPC), np.float32)
        ohx[hi, np.arange(RPC)] = 1.0
        ohy = np.zeros((E, RPC), np.float32)
        ohy[ti, np.arange(RPC)] = 1.0
        # exact-scaled gather: x columns are 0/1, y columns are 0 or 1/64
        # (both exact in fp8/fp16); the per-pair 1/(cntx*cnty) mean factor
        # cancels in the rel normalization, with the 1e-5 eps rescaled by
        # epsb = 16e-5 * cntx * cnty / 64 per pair
        ohxs = (S @ ohx)            # [M, RPC] in {0, 1}
        ohys = (S @ ohy) * (1.0 / 64.0)
        cnt1 = np.maximum(cnt, 1.0)
        scl = cnt1[hi] * cnt1[ti] / 64.0    # [RPC]
        epsb = np.ascontiguousarray(
            (16e-5 * scl).reshape(2, 128).T.astype(np.float32))  # [128, 2]
        ohxy2 = np.concatenate([ohxs, ohys], axis=1)  # [M, 512]
        a = (ent_to_seq_attn[doc].transpose(1, 0, 2)  # [M, NH, L]
             .reshape(M, NH, 8, 128).transpose(0, 2, 1, 3))  # [M, 8, NH, 128]
        if P3_FP8:
            f8 = _np_fp8()
            # m = t*64 + k  ->  [k(64), t(2), lc, h, f]
            attn_c = np.ascontiguousarray(
                a.reshape(2, 64, 8, NH, 128).transpose(1, 0, 2, 3, 4)
                .reshape(64, 2 * 8 * NH * 128)).astype(f8)
            ohxy2_c = np.ascontiguousarray(
                ohxy2.reshape(2, 64, 2 * RPC).transpose(1, 0, 2)
                .reshape(64, 2 * 2 * RPC)).astype(f8)
        else:
            attn_c = np.ascontiguousarray(
                a.reshape(M, 8 * NH * 128)).astype(fdt)
            ohxy2_c = ohxy2.astype(fdt)
        seq_r = seq_lhs[doc].reshape(8, 128, L).transpose(1, 0, 2)
        seq_aug = np.concatenate(
            [seq_r, np.ones((128, 8, 1), np.float32)], axis=2
        )
        in_maps.append({
            "ent": np.ascontiguousarray(ent_lhs[doc]),
            "attn": attn_c,
            "seq": np.ascontiguousarray(
                seq_aug.reshape(128, 8 * (L + 1))).astype(fdt),
            "ssum": S.astype(fdt),
            "ohxy2": ohxy2_c,
            "epsb": epsb,
            "eadd": eadd,
            "ohx": ohx.astype(fdt),
            "ohy": ohy.astype(fdt),
            "wh": wh_c, "wt": wt_c, "bh": bh_c, "bt": bt_c,
            "wb": wb_r, "bbc": bb_c, "ident": ident, "repm": repm_h,
        })
    return in_maps


_LAST_RESULTS = {}


def kernel(**inputs) -> np.ndarray:
    prog = _get_program()
    in_maps = _host_inputs(**inputs)
    trace = os.environ.get("DOCRED_TRACE", "0") == "1"
    res = run_bass_kernel_spmd(
        prog.nc, in_maps, core_ids=list(range(NCORES)), trace=trace,
    )
    _LAST_RESULTS["res"] = res
    out = np.empty((B * R, NCL), np.float32)
    for c in range(NCORES):
        doc, half = divmod(c, 2)
        lt = res.results[c]["lt"]  # [NCL, RPC]
        out[doc * R + half * RPC: doc * R + (half + 1) * RPC, :] = lt.T
    return out


# revision 40
# speedup vs baseline: 1.1670x; 1.0748x over previous
"""Trainium2 Bass kernel for the DocRED-style segment_reduce model.

Sharding: 8 cores, data-parallel: core c -> (doc = c//2, pair-half = c%2).
Each core independently computes logits for its 256 pairs. No collectives.
All segment reductions / gathers are lowered to one-hot matmuls whose
one-hot matrices are built on the host from the integer inputs and passed
as per-core input tensors (the SPMD program itself is index-agnostic).

Key facts driving the design (measured on HW):
  - the PE throttles to ~50% utilization under sustained load
    (throttle_activity_1_avg_util_limit=0.5), so total PE cycles are the
    main budget; short bursts run at 2.4 GHz, sustained streams ~1.2 GHz
  - DVE/GPSIMD can read at most one (DVE) / zero (GPSIMD) PSUM operands,
    so every PE result used by a product must be drained by ACT first
  - the attention-gather one-hots are EXACT 0/1 (or 1/64) matrices: the
    segment-mean scale cancels in the rel normalization, so the gather
    matmuls can run in fp8e4m3 DoubleRow (2x PE throughput) with only
    the attention values themselves quantized (DOCRED_P3_FP8=1)
"""

import os

import numpy as np

import concourse.bacc as bacc
import concourse.bass as bass
import concourse.mybir as mybir
import concourse.tile as tile
from concourse.bass_utils import run_bass_kernel_spmd

B, M, H = 4, 128, 1024
NH, L = 16, 1024
E, R = 64, 512
EMB, BS, NCL = 768, 64, 97
K12 = EMB // BS  # 12 blocks
NCORES = 8
RPC = R // 2  # pairs per core

F32 = mybir.dt.float32
F16 = mybir.dt.float16
BF16 = mybir.dt.bfloat16
FP8 = mybir.dt.float8e4

MM_MODE = os.environ.get("DOCRED_MM_MODE", "f16")
# P3 gather matmuls in fp8e4m3 DoubleRow (2x PE throughput)
P3_FP8 = os.environ.get("DOCRED_P3_FP8", "0") == "1"
# P3 per-pack product route, one per 2-head pack (8 packs per lc)
# ("a": ACT copies x-half, DVE mults SBUF x PSUM; "b": ACT copies both,
# DVE fp16; "g": ACT copies both, GpSimd mults)
P3_ROUTES = os.environ.get("DOCRED_P3_ROUTES", "a,a,g,a,a,a,g,a").split(",")
# P6 k-block routes ("pa": PE one-hot replication + ACT copy (even k only);
# "pd": PE replication, DVE mults from PSUM; "d": DMA broadcast replication)
P6_ROUTES = os.environ.get(
    "DOCRED_P6_ROUTES", "pa,d,d,d,d,d,d,d,d,d,d,d").split(",")

_FULL_KEY = (MM_MODE, P3_FP8, tuple(P3_ROUTES), tuple(P6_ROUTES))


def _fdt():
    return BF16 if MM_MODE == "bf16" else F16


def _np_fdt():
    import ml_dtypes

    return np.dtype(ml_dtypes.bfloat16) if MM_MODE == "bf16" else np.float16


def _np_fp8():
    import ml_dtypes

    return np.dtype(ml_dtypes.float8_e4m3)


class _Builder:
    def __init__(self):
        self.fdt = _fdt()
        nc = bacc.Bacc("TRN2", target_bir_lowering=False, debug=False)
        self.nc = nc
        fdt = self.fdt
        d = {}
        d["ent"] = nc.dram_tensor("ent", [M, H], F32, kind="ExternalInput")
        if P3_FP8:
            # [k(64), t(2), lc, h, 128] fp8 (m = t*64 + k)
            d["attn"] = nc.dram_tensor("attn", [64, 2 * 8 * NH * 128], FP8,
                                       kind="ExternalInput")
            d["ohxy2"] = nc.dram_tensor("ohxy2", [64, 2 * 2 * RPC], FP8,
                                        kind="ExternalInput")
        else:
            d["attn"] = nc.dram_tensor("attn", [M, 8 * NH * 128], fdt,
                                       kind="ExternalInput")
            d["ohxy2"] = nc.dram_tensor("ohxy2", [M, 2 * RPC], fdt,
                                        kind="ExternalInput")
        d["epsb"] = nc.dram_tensor("epsb", [128, 2], F32, kind="ExternalInput")
        d["seq"] = nc.dram_tensor("seq", [128, 8 * (L + 1)], fdt,
                                  kind="ExternalInput")
        d["ssum"] = nc.dram_tensor("ssum", [M, E], fdt, kind="ExternalInput")
        d["eadd"] = nc.dram_tensor("eadd", [E, 1], F32, kind="ExternalInput")
        d["ohx"] = nc.dram_tensor("ohx", [E, RPC], fdt, kind="ExternalInput")
        d["ohy"] = nc.dram_tensor("ohy", [E, RPC], fdt, kind="ExternalInput")
        d["wh"] = nc.dram_tensor("wh", [128, 16 * EMB], fdt, kind="ExternalInput")
        d["wt"] = nc.dram_tensor("wt", [128, 16 * EMB], fdt, kind="ExternalInput")
        d["bh"] = nc.dram_tensor("bh", [128, EMB // 128], F32, kind="ExternalInput")
        d["bt"] = nc.dram_tensor("bt", [128, EMB // 128], F32, kind="ExternalInput")
        d["wb"] = nc.dram_tensor("wb", [128, 384 * NCL], fdt, kind="ExternalInput")
        d["bbc"] = nc.dram_tensor("bbc", [NCL, 1], F32, kind="ExternalInput")
        d["ident"] = nc.dram_tensor("ident", [128, 128], fdt, kind="ExternalInput")
        d["repm"] = nc.dram_tensor("repm", [E, 32 * 128], fdt, kind="ExternalInput")
        d["lt"] = nc.dram_tensor("lt", [NCL, RPC], F32, kind="ExternalOutput")
        self.d = d
        with tile.TileContext(nc) as tc:
            self.build(tc)
        nc.compile()

    def mm(self, out, lhsT, rhs, **kw):
        return self.nc.tensor.matmul(out, lhsT, rhs, **kw)

    def tp(self, out, in_, ident, **kw):
        return self.nc.tensor.matmul(out, in_, ident, is_transpose=True, **kw)

    def build(self, tc):
        nc = self.nc
        d = self.d
        fdt = self.fdt
        AF = mybir.ActivationFunctionType
        DR = mybir.MatmulPerfMode.DoubleRow

        with (
            tc.tile_pool(name="pin", bufs=1) as pin,
            tc.tile_pool(name="mid", bufs=1) as mid,
            tc.tile_pool(name="prodp", bufs=2) as prodp,
            tc.tile_pool(name="dramp", bufs=1, space="DRAM") as dramp,
        ):
            # phase-nested pools: all three close after P5, then the P6
            # pool reuses their space
            attn_cm = tc.tile_pool(name="attnp", bufs=1)
            attnp = attn_cm.__enter__()
            seq_cm = tc.tile_pool(name="seqp", bufs=1)
            seqp = seq_cm.__enter__()
            wpin_cm = tc.tile_pool(name="wpin", bufs=1)
            wpin = wpin_cm.__enter__()

            # ---------- tiles ----------
            ident = pin.tile([128, 128], fdt)
            ssum = pin.tile([M, E], fdt)
            eadd = pin.tile([E, 1], F32)
            epsb = pin.tile([128, 2], F32)
            ohx = pin.tile([E, RPC], fdt)
            ohy = pin.tile([E, RPC], fdt)
            bh = pin.tile([128, EMB // 128], F32)
            bt = pin.tile([128, EMB // 128], F32)
            bbc = pin.tile([NCL, 1], F32)
            repm = pin.tile([E, 32, 128], fdt)
            ent = mid.tile([M, H], F32)
            if P3_FP8:
                attn = attnp.tile([64, 2, 8, NH, 128], FP8)
                ohxy2 = pin.tile([64, 2, 2 * RPC], FP8)
                av = d["attn"].ap().rearrange(
                    "p (t lc h f) -> p t lc h f", t=2, lc=8, h=NH)
                ov = d["ohxy2"].ap().rearrange("p (t n) -> p t n", t=2)
            else:
                attn = attnp.tile([M, 8, NH, 128], fdt)
                ohxy2 = pin.tile([M, 2 * RPC], fdt)
                av = d["attn"].ap().rearrange(
                    "p (lc h f) -> p lc h f", lc=8, h=NH)
                ov = d["ohxy2"].ap()
            sq = seqp.tile([128, 8, L + 1], fdt)
            wh_sb = wpin.tile([128, 16, EMB], fdt, name="wh_sb")
            wt_sb = wpin.tile([128, 16, EMB], fdt, name="wt_sb")

            # ---------- DMA priority order ----------
            nc.sync.dma_start(ent[:], d["ent"].ap())
            for t, key in [(ssum, "ssum"), (eadd, "eadd")]:
                nc.sync.dma_start(t[:], d[key].ap())
            nc.sync.dma_start(ohxy2[:], ov)
            for t, key in [
                (ident, "ident"), (ohx, "ohx"), (ohy, "ohy"), (epsb, "epsb"),
                (bh, "bh"), (bt, "bt"), (bbc, "bbc"),
            ]:
                nc.sync.dma_start(t[:], d[key].ap())
            if P3_FP8:
                for lc in range(8):
                    nc.sync.dma_start(attn[:, :, lc], av[:, :, lc])
            else:
                for lc in range(8):
                    nc.sync.dma_start(attn[:, lc], av[:, lc])
            nc.sync.dma_start(
                wh_sb[:], d["wh"].ap().rearrange("p (a b) -> p a b", a=16))
            nc.sync.dma_start(
                wt_sb[:], d["wt"].ap().rearrange("p (a b) -> p a b", a=16))
            nc.sync.dma_start(sq[:], d["seq"].ap()
                              .rearrange("p (a b) -> p a b", a=8))
            nc.scalar.dma_start(repm[:], d["repm"].ap()
                                .rearrange("p (a b) -> p a b", a=32))

            # wb stream: prefetch first 4 chunks now (scalar queue) into
            # persistent tiles; the rest rotate through the P6 pool
            wb_tiles = {}
            for k in range(4):
                wb = mid.tile([128, 32 * NCL], fdt, name=f"wbp{k}")
                nc.sync.dma_start(
                    wb[:], d["wb"].ap()[:, k * 32 * NCL:(k + 1) * 32 * NCL])
                wb_tiles[k] = wb

            # ---------- P3 + P1 ----------
            # P1's matmuls are issued between lc0 and lc1 so the PE queue
            # never stalls on the (slow to start) ACT exp; ps_ent shares
            # the P3 PSUM pool to avoid a pool barrier.
            pexp = mid.tile([M, H], fdt, name="pexp")
            nc.scalar.activation(pexp[:], ent[:], AF.Exp)
            ent_sb = mid.tile([E, H], fdt)

            CTmm = mid.tile([128, 8, RPC], fdt, name="CTmm")
            psP_cm = tc.tile_pool(name="psP", bufs=1, space="PSUM")
            psP = psP_cm.__enter__()
            ps_ent = psP.tile([E, H], F32, tag="ent", bufs=1, name="ps_ent")
            for lc in range(8):
                prods = []
                for q in range(8):
                    psq = psP.tile([128, 2, 2, RPC], F32, tag="p3", bufs=3,
                                   name="psq")
                    for hh in range(2):
                        h = q * 2 + hh
                        if P3_FP8:
                            self.mm(psq[:, hh], attn[:, :, lc, h, :],
                                    ohxy2[:], perf_mode=DR)
                        else:
                            self.mm(psq[:, hh], attn[:, lc, h, :], ohxy2[:])
                    prod = prodp.tile([128, 2, RPC], fdt, tag=f"prod{q % 4}",
                                      bufs=4, name=f"prod{q % 4}")
                    route = P3_ROUTES[q]
                    if route in ("b", "g"):
                        g16 = prodp.tile([128, 2, 2, RPC], fdt, tag="g16",
                                         bufs=2, name="g16")
                        nc.scalar.copy(g16[:], psq[:])
                        eng = nc.gpsimd if route == "g" else nc.vector
                        eng.tensor_mul(prod[:], g16[:, :, 0, :],
                                       g16[:, :, 1, :])
                    else:
                        gx = prodp.tile([128, 2, RPC], fdt, tag="gx",
                                        bufs=3, name="gx")
                        nc.scalar.copy(gx[:], psq[:, :, 0, :])
                        nc.vector.tensor_mul(prod[:], gx[:], psq[:, :, 1, :])
                    prods.append(prod)
                if lc == 0:
                    # P1 segment-sum matmuls ride the PE queue here
                    for nh in range(2):
                        self.mm(ps_ent[:, nh * 512:(nh + 1) * 512], ssum[:],
                                pexp[:, nh * 512:(nh + 1) * 512])
                    nc.scalar.activation(ent_sb[:], ps_ent[:], AF.Ln,
                                         bias=eadd[:])
                # tree reduce 8 tiles -> CTmm[:, lc, :]
                for st in (0, 4):
                    nc.vector.tensor_add(prods[st][:], prods[st][:],
                                         prods[st + 1][:])
                    nc.vector.tensor_add(prods[st + 2][:], prods[st + 2][:],
                                         prods[st + 3][:])
                    nc.vector.tensor_add(prods[st][:], prods[st][:],
                                         prods[st + 2][:])
                nc.vector.tensor_add(prods[0][:], prods[0][:], prods[4][:])
                nc.vector.tensor_add(CTmm[:, lc, :], prods[0][:, 0, :],
                                     prods[0][:, 1, :])
            psP_cm.__exit__(None, None, None)

            # entT: [h-part, hc, e] (issued after P3 so the PE queue never
            # stalls on the ident DMA)
            psT_cm = tc.tile_pool(name="psT", bufs=2, space="PSUM")
            psT = psT_cm.__enter__()
            entT = mid.tile([128, 8, E], fdt, name="entT")
            for hc in range(8):
                ps_t2 = psT.tile([128, E], fdt, tag="tp")
                self.tp(ps_t2[:], ent_sb[:, hc * 128:(hc + 1) * 128],
                        ident[0:E, 0:E])
                nc.scalar.copy(entT[:, hc, :], ps_t2[:])
            psT_cm.__exit__(None, None, None)

            # ---------- EW = ent_sb @ W[0:1024] (fills the P3->P4 gap) ----
            psEW_cm = tc.tile_pool(name="psEW", bufs=2, space="PSUM")
            psEW = psEW_cm.__enter__()
            EWh = mid.tile([E, EMB], fdt, name="EWh")
            EWt = mid.tile([E, EMB], fdt, name="EWt")
            for w, ew in ((wh_sb, EWh), (wt_sb, EWt)):
                ps_ew = psEW.tile([E, EMB], F32, tag="ew")
                for hc in range(8):
                    for lo, hi in ((0, 512), (512, 768)):
                        self.mm(ps_ew[:, lo:hi], entT[:, hc, :],
                                w[:, hc, lo:hi],
                                start=(hc == 0), stop=(hc == 7))
                nc.scalar.copy(ew[:], ps_ew[:])
            psEW_cm.__exit__(None, None, None)

            # ---------- P4: rel = normalize(C) @ seq ----------
            psR_cm = tc.tile_pool(name="psR", bufs=1, space="PSUM")
            psR = psR_cm.__enter__()
            ps_rel = [psR.tile([128, L], F32, name=f"ps_rel{i}")
                      for i in range(2)]
            ps_s8 = psR.tile([128, 2, 8], F32, name="ps_s8")
            for lc in range(8):
                st, sp = lc == 0, lc == 7
                for rc in range(2):
                    lhsT = CTmm[:, lc, rc * 128:(rc + 1) * 128]
                    self.mm(ps_rel[rc][:, 0:512], lhsT, sq[:, lc, 0:512],
                            start=st, stop=sp)
                    self.mm(ps_rel[rc][:, 512:1024], lhsT, sq[:, lc, 512:1024],
                            start=st, stop=sp)
                    self.mm(ps_s8[:, rc, lc:lc + 1], lhsT,
                            sq[:, lc, 1024:1025], start=True, stop=True)
            relT = mid.tile([128, 8, RPC], fdt, name="relT")
            psT2_cm = tc.tile_pool(name="psT2", bufs=2, space="PSUM")
            psT2 = psT2_cm.__enter__()
            for rc in range(2):
                tdenom = prodp.tile([128, 1], F32, tag="tden")
                nc.vector.tensor_reduce(tdenom[:], ps_s8[:, rc, :],
                                        axis=mybir.AxisListType.X,
                                        op=mybir.AluOpType.add)
                nc.scalar.activation(tdenom[:], tdenom[:], AF.Identity,
                                     bias=epsb[:, rc:rc + 1], scale=1.0)
                frec = prodp.tile([128, 1], F32, tag="frec")
                nc.vector.reciprocal(frec[:], tdenom[:])
                rel_sc = mid.tile([128, L], fdt, tag="rel_sc", name="rel_sc")
                nc.vector.tensor_scalar_mul(rel_sc[:], ps_rel[rc][:], frec[:])
                for dc in range(8):
                    ps_t = psT2.tile([128, 128], fdt, tag="tp2")
                    self.tp(ps_t[:], rel_sc[:, dc * 128:(dc + 1) * 128],
                            ident[:])
                    nc.scalar.copy(relT[:, dc, rc * 128:(rc + 1) * 128],
                                   ps_t[:])
            psT2_cm.__exit__(None, None, None)
            psR_cm.__exit__(None, None, None)

            # ---------- P5: extractors -> hsEt/tsEt [emb, n] ----------
            psE_cm = tc.tile_pool(name="psE", bufs=4, space="PSUM")
            psE = psE_cm.__enter__()
            hsEt = mid.tile([128, 6, RPC], fdt, name="hsEt")
            tsEt = mid.tile([128, 6, RPC], fdt, name="tsEt")
            # ec-major staging: the P6 broadcast reads one contiguous 16KB
            # block per partition
            hsd = dramp.tile([6, 128, RPC], fdt, name="hsd")
            tsd = dramp.tile([6, 128, RPC], fdt, name="tsd")
            for ec in range(6):
                for (w, bvec, ew, oh, dst, dstd) in (
                    (wh_sb, bh, EWh, ohx, hsEt, hsd),
                    (wt_sb, bt, EWt, ohy, tsEt, tsd),
                ):
                    ps_e = psE.tile([128, RPC], F32, tag="pe", name="ps_e")
                    self.mm(ps_e[:], ew[:, ec * 128:(ec + 1) * 128], oh[:],
                            start=True, stop=False)
                    for kc in range(8, 16):
                        self.mm(ps_e[:], w[:, kc, ec * 128:(ec + 1) * 128],
                                relT[:, kc % 8, :],
                                start=False, stop=(kc == 15))
                    nc.scalar.activation(dst[:, ec, :], ps_e[:], AF.Tanh,
                                         bias=bvec[:, ec:ec + 1])
                    nc.sync.dma_start(dstd[ec], dst[:, ec, :])
            psE_cm.__exit__(None, None, None)
            wpin_cm.__exit__(None, None, None)
            seq_cm.__exit__(None, None, None)
            attn_cm.__exit__(None, None, None)

            # ---------- P6: block bilinear + classifier ----------
            with (
                tc.tile_pool(name="blph", bufs=1) as blph,
                tc.tile_pool(name="ps_lt", bufs=1, space="PSUM") as ps_lt,
                tc.tile_pool(name="psRep", bufs=2, space="PSUM") as psRep,
            ):
                pslt = ps_lt.tile([NCL, RPC], F32)

                def issue_b2t(k, tag="b2t", bufs=3):
                    kk = 64 * (k % 2)
                    ec = k // 2
                    b2t = blph.tile([128, RPC], fdt, tag=tag, bufs=bufs,
                                    name=tag)
                    for h0 in (0, 1):
                        nc.sync.dma_start(b2t[64 * h0:64 * (h0 + 1)],
                                          tsd[ec, kk:kk + 64, :])
                    return b2t

                def issue_b1rep(k, bufs=3, tag="b1rep"):
                    # the replication broadcasts get their own DMA queue
                    # (scalar) so the wb/b2t stream never sits behind them
                    kk = 64 * (k % 2)
                    ec = k // 2
                    b1rep = blph.tile([128, 32, RPC], fdt, tag=tag, bufs=bufs,
                                      name=tag)
                    for h0 in (0, 1):
                        src = hsd[ec, kk + 32 * h0:kk + 32 * (h0 + 1), :] \
                            .unsqueeze(0).broadcast_to([64, 32, RPC])
                        nc.scalar.dma_start(b1rep[64 * h0:64 * (h0 + 1)], src)
                    return b1rep

                cg = 0
                for k in range(K12):
                    kk = 64 * (k % 2)
                    ec = k // 2
                    route = P6_ROUTES[k]
                    if k in wb_tiles:
                        wb = wb_tiles[k]
                    else:
                        wb = blph.tile([128, 32 * NCL], fdt, tag="wb",
                                       bufs=3, name="wb")
                        nc.sync.dma_start(
                            wb[:],
                            d["wb"].ap()[:, k * 32 * NCL:(k + 1) * 32 * NCL])
                    b2t = issue_b2t(k)
                    blT = blph.tile([128, 32, RPC], fdt, tag="blT",
                                    bufs=2, name="blT")
                    if route in ("pd", "pa"):
                        assert kk == 0, "PE replication route needs even k"
                        hsE64 = hsEt[kk:kk + 64, ec, :]
                        b2b = b2t[:].unsqueeze(1).broadcast_to([128, 4, RPC])
                        for cq in range(8):
                            psq6 = psRep.tile([128, 4, RPC], F32, tag="rep",
                                              bufs=2, name="psq6")
                            for i4 in range(4):
                                self.mm(psq6[:, i4, :],
                                        repm[:, cq * 4 + i4, :], hsE64)
                            if route == "pa":
                                b1c = blph.tile([128, 4, RPC], fdt, tag="b1c",
                                                bufs=3, name="b1c")
                                nc.scalar.copy(b1c[:], psq6[:])
                                nc.vector.tensor_mul(
                                    blT[:, cq * 4:(cq + 1) * 4, :],
                                    b1c[:], b2b)
                            else:
                                nc.vector.tensor_mul(
                                    blT[:, cq * 4:(cq + 1) * 4, :],
                                    psq6[:], b2b)
                    else:
                        b1rep = issue_b1rep(k)
                        b2b = b2t[:].unsqueeze(1).broadcast_to([128, 8, RPC])
                        for g in range(4):
                            nc.vector.tensor_mul(
                                blT[:, g * 8:(g + 1) * 8, :],
                                b1rep[:, g * 8:(g + 1) * 8, :], b2b)
                    for c in range(32):
                        self.mm(pslt[:], wb[:, c * NCL:(c + 1) * NCL],
                                blT[:, c, :],
                                start=(cg == 0), stop=(cg == 383))
                        cg += 1

                out_sb = mid.tile([NCL, RPC], F32)
                nc.scalar.activation(out_sb[:], pslt[:], AF.Identity,
                                     bias=bbc[:])
                nc.sync.dma_start(d["lt"].ap(), out_sb[:])


_PROGRAM_CACHE = {}


def _get_program():
    if _FULL_KEY not in _PROGRAM_CACHE:
        _PROGRAM_CACHE[_FULL_KEY] = _Builder()
    return _PROGRAM_CACHE[_FULL_KEY]


def _host_inputs(seq_lhs, ent_lhs, ent_to_seq_attn, entity_id_labels, hts,
                 Wh, bh, Wt, bt, Wb, bb):
    """Build the 8 per-core input maps (all host-side numpy)."""
    fdt = _np_fdt()
    seq_lhs = np.asarray(seq_lhs, np.float32)
    ent_lhs = np.asarray(ent_lhs, np.float32)
    ent_to_seq_attn = np.asarray(ent_to_seq_attn, np.float32)
    entity_id_labels = np.asarray(entity_id_labels)
    hts = np.asarray(hts)
    Wh = np.asarray(Wh, np.float32)
    Wt = np.asarray(Wt, np.float32)
    Wb = np.asarray(Wb, np.float32)
    bh = np.asarray(bh, np.float32)
    bt = np.asarray(bt, np.float32)
    bb = np.asarray(bb, np.float32)

    # device chunk (k, c) row p maps to Wb row k*4096 + i*64 + j with
    # i = c + 32*(p//64), j = p%64
    p_ = np.arange(128)
    c_ = np.arange(32)
    k_ = np.arange(K12)
    rows = (k_[:, None, None] * 4096
            + (c_[None, :, None] + 32 * (p_[None, None, :] // 64)) * 64
            + (p_[None, None, :] % 64))  # [k, c, p]
    wb_r = np.ascontiguousarray(
        Wb[rows.reshape(-1), :].reshape(K12 * 32, 128, NCL)
        .transpose(1, 0, 2).reshape(128, 384 * NCL)
    ).astype(fdt)
    wh_c = np.ascontiguousarray(
        Wh.reshape(16, 128, EMB).transpose(1, 0, 2).reshape(128, 16 * EMB)
    ).astype(fdt)
    wt_c = np.ascontiguousarray(
        Wt.reshape(16, 128, EMB).transpose(1, 0, 2).reshape(128, 16 * EMB)
    ).astype(fdt)
    bh_c = np.ascontiguousarray(bh.reshape(EMB // 128, 128).T)
    bt_c = np.ascontiguousarray(bt.reshape(EMB // 128, 128).T)
    bb_c = np.ascontiguousarray(bb.reshape(NCL, 1))
    ident = np.eye(128, dtype=np.float32).astype(fdt)
    # repm[r, c, p] = 1 iff r == c + 32*(p//64)
    repm_h = np.zeros((E, 32, 128), np.float32)
    for c in range(32):
        repm_h[c, c, 0:64] = 1.0
        repm_h[c + 32, c, 64:128] = 1.0
    repm_h = repm_h.reshape(E, 32 * 128).astype(fdt)

    in_maps = []
    for c in range(NCORES):
        doc, half = divmod(c, 2)
        sl = slice(half * RPC, (half + 1) * RPC)
        labels = entity_id_labels[doc].astype(np.int64)
        cnt = np.bincount(labels, minlength=E).astype(np.float32)
        S = np.zeros((M, E), np.float32)
        S[np.arange(M), labels] = 1.0
        eadd = (cnt == 0).astype(np.float32).reshape(E, 1)
        hi = hts[doc, sl, 0].astype(np.int64)
        ti = hts[doc, sl, 1].astype(np.int64)
        ohx = np.zeros((E, RPC), np.float32)
        ohx[hi, np.arange(RPC)] = 1.0
        ohy = np.zeros((E, RPC), np.float32)
        ohy[ti, np.arange(RPC)] = 1.0
        # exact-scaled gather: x columns are 0/1, y columns are 0 or 1/64
        # (both exact in fp8/fp16); the per-pair 1/(cntx*cnty) mean factor
        # cancels in the rel normalization, with the 1e-5 eps rescaled by
        # epsb = 16e-5 * cntx * cnty / 64 per pair
        ohxs = (S @ ohx)            # [M, RPC] in {0, 1}
        ohys = (S @ ohy) * (1.0 / 64.0)
        cnt1 = np.maximum(cnt, 1.0)
        scl = cnt1[hi] * cnt1[ti] / 64.0    # [RPC]
        epsb = np.ascontiguousarray(
            (16e-5 * scl).reshape(2, 128).T.astype(np.float32))  # [128, 2]
        ohxy2 = np.concatenate([ohxs, ohys], axis=1)  # [M, 512]
        a = (ent_to_seq_attn[doc].transpose(1, 0, 2)  # [M, NH, L]
             .reshape(M, NH, 8, 128).transpose(0, 2, 1, 3))  # [M, 8, NH, 128]
        if P3_FP8:
            f8 = _np_fp8()
            # m = t*64 + k  ->  [k(64), t(2), lc, h, f]
            attn_c = np.ascontiguousarray(
                a.reshape(2, 64, 8, NH, 128).transpose(1, 0, 2, 3, 4)
                .reshape(64, 2 * 8 * NH * 128)).astype(f8)
            ohxy2_c = np.ascontiguousarray(
                ohxy2.reshape(2, 64, 2 * RPC).transpose(1, 0, 2)
                .reshape(64, 2 * 2 * RPC)).astype(f8)
        else:
            attn_c = np.ascontiguousarray(
                a.reshape(M, 8 * NH * 128)).astype(fdt)
            ohxy2_c = ohxy2.astype(fdt)
        seq_r = seq_lhs[doc].reshape(8, 128, L).transpose(1, 0, 2)
        seq_aug = np.concatenate(
            [seq_r, np.ones((128, 8, 1), np.float32)], axis=2
        )
        in_maps.append({
            "ent": np.ascontiguousarray(ent_lhs[doc]),
            "attn": attn_c,
            "seq": np.ascontiguousarray(
                seq_aug.reshape(128, 8 * (L + 1))).astype(fdt),
            "ssum": S.astype(fdt),
            "ohxy2": ohxy2_c,
            "epsb": epsb,
            "eadd": eadd,
            "ohx": ohx.astype(fdt),
            "ohy": ohy.astype(fdt),
            "wh": wh_c, "wt": wt_c, "bh": bh_c, "bt": bt_c,
            "wb": wb_r, "bbc": bb_c, "ident": ident, "repm": repm_h,
        })
    return in_maps


_LAST_RESULTS = {}


def kernel(**inputs) -> np.ndarray:
    prog = _get_program()
    in_maps = _host_inputs(**inputs)
    trace = os.environ.get("DOCRED_TRACE", "0") == "1"
    res = run_bass_kernel_spmd(
        prog.nc, in_maps, core_ids=list(range(NCORES)), trace=trace,
    )
    _LAST_RESULTS["res"] = res
    out = np.empty((B * R, NCL), np.float32)
    for c in range(NCORES):
        doc, half = divmod(c, 2)
        lt = res.results[c]["lt"]  # [NCL, RPC]
        out[doc * R + half * RPC: doc * R + (half + 1) * RPC, :] = lt.T
    return out


# revision 42
# speedup vs baseline: 1.2695x; 1.0878x over previous
"""Trainium2 Bass kernel for the DocRED-style segment_reduce model.

Sharding: 8 cores, data-parallel: core c -> (doc = c//2, pair-half = c%2).
Each core independently computes logits for its 256 pairs. No collectives.
All segment reductions / gathers are lowered to one-hot matmuls whose
one-hot matrices are built on the host from the integer inputs and passed
as per-core input tensors (the SPMD program itself is index-agnostic).

Key facts driving the design (measured on HW):
  - the PE throttles to ~50% utilization under sustained load
    (throttle_activity_1_avg_util_limit=0.5), so total PE cycles are the
    main budget; short bursts run at 2.4 GHz, sustained streams ~1.2 GHz
  - DVE/GPSIMD can read at most one (DVE) / zero (GPSIMD) PSUM operands,
    so every PE result used by a product must be drained by ACT first
  - the attention-gather one-hots are EXACT 0/1 (or 1/64) matrices: the
    segment-mean scale cancels in the rel normalization, so the gather
    matmuls can run in fp8e4m3 DoubleRow (2x PE throughput) with only
    the attention values themselves quantized (DOCRED_P3_FP8=1)
"""

import os

import numpy as np

import concourse.bacc as bacc
import concourse.bass as bass
import concourse.mybir as mybir
import concourse.tile as tile
from concourse.bass_utils import run_bass_kernel_spmd

B, M, H = 4, 128, 1024
NH, L = 16, 1024
E, R = 64, 512
EMB, BS, NCL = 768, 64, 97
K12 = EMB // BS  # 12 blocks
NCORES = 8
RPC = R // 2  # pairs per core

F32 = mybir.dt.float32
F16 = mybir.dt.float16
BF16 = mybir.dt.bfloat16
FP8 = mybir.dt.float8e4

MM_MODE = os.environ.get("DOCRED_MM_MODE", "f16")
# P3 gather matmuls in fp8e4m3 DoubleRow (2x PE throughput)
P3_FP8 = os.environ.get("DOCRED_P3_FP8", "0") == "1"
# P3 per-pack product route, one per 2-head pack (8 packs per lc)
# ("a": ACT copies x-half, DVE mults SBUF x PSUM; "b": ACT copies both,
# DVE fp16; "g": ACT copies both, GpSimd mults)
P3_ROUTES = os.environ.get("DOCRED_P3_ROUTES", "a,a,g,a,a,a,g,a").split(",")
# P6 k-block routes ("pa": PE one-hot replication + ACT copy (even k only);
# "pd": PE replication, DVE mults from PSUM; "d": DMA broadcast replication)
P6_ROUTES = os.environ.get(
    "DOCRED_P6_ROUTES", "pa,d,d,d,pa,d,d,d,pa,d,d,d").split(",")

_FULL_KEY = (MM_MODE, P3_FP8, tuple(P3_ROUTES), tuple(P6_ROUTES))


def _fdt():
    return BF16 if MM_MODE == "bf16" else F16


def _np_fdt():
    import ml_dtypes

    return np.dtype(ml_dtypes.bfloat16) if MM_MODE == "bf16" else np.float16


def _np_fp8():
    import ml_dtypes

    return np.dtype(ml_dtypes.float8_e4m3)


class _Builder:
    def __init__(self):
        self.fdt = _fdt()
        nc = bacc.Bacc("TRN2", target_bir_lowering=False, debug=False)
        self.nc = nc
        fdt = self.fdt
        d = {}
        d["ent"] = nc.dram_tensor("ent", [M, H], F32, kind="ExternalInput")
        if P3_FP8:
            # [k(64), t(2), lc, h, 128] fp8 (m = t*64 + k)
            d["attn"] = nc.dram_tensor("attn", [64, 2 * 8 * NH * 128], FP8,
                                       kind="ExternalInput")
            d["ohxy2"] = nc.dram_tensor("ohxy2", [64, 2 * 2 * RPC], FP8,
                                        kind="ExternalInput")
        else:
            d["attn"] = nc.dram_tensor("attn", [M, 8 * NH * 128], fdt,
                                       kind="ExternalInput")
            d["ohxy2"] = nc.dram_tensor("ohxy2", [M, 2 * RPC], fdt,
                                        kind="ExternalInput")
        d["epsb"] = nc.dram_tensor("epsb", [128, 2], F32, kind="ExternalInput")
        d["seq"] = nc.dram_tensor("seq", [128, 8 * (L + 1)], fdt,
                                  kind="ExternalInput")
        d["ssum"] = nc.dram_tensor("ssum", [M, E], fdt, kind="ExternalInput")
        d["eadd"] = nc.dram_tensor("eadd", [E, 1], F32, kind="ExternalInput")
        d["ohx"] = nc.dram_tensor("ohx", [E, RPC], fdt, kind="ExternalInput")
        d["ohy"] = nc.dram_tensor("ohy", [E, RPC], fdt, kind="ExternalInput")
        d["wh"] = nc.dram_tensor("wh", [128, 16 * EMB], fdt, kind="ExternalInput")
        d["wt"] = nc.dram_tensor("wt", [128, 16 * EMB], fdt, kind="ExternalInput")
        d["bh"] = nc.dram_tensor("bh", [128, EMB // 128], F32, kind="ExternalInput")
        d["bt"] = nc.dram_tensor("bt", [128, EMB // 128], F32, kind="ExternalInput")
        d["wb"] = nc.dram_tensor("wb", [128, 384 * NCL], fdt, kind="ExternalInput")
        d["bbc"] = nc.dram_tensor("bbc", [NCL, 1], F32, kind="ExternalInput")
        d["ident"] = nc.dram_tensor("ident", [128, 128], fdt, kind="ExternalInput")
        d["repm"] = nc.dram_tensor("repm", [E, 32 * 128], fdt, kind="ExternalInput")
        d["lt"] = nc.dram_tensor("lt", [NCL, RPC], F32, kind="ExternalOutput")
        self.d = d
        with tile.TileContext(nc) as tc:
            self.build(tc)
        nc.compile()

    def mm(self, out, lhsT, rhs, **kw):
        return self.nc.tensor.matmul(out, lhsT, rhs, **kw)

    def tp(self, out, in_, ident, **kw):
        return self.nc.tensor.matmul(out, in_, ident, is_transpose=True, **kw)

    def build(self, tc):
        nc = self.nc
        d = self.d
        fdt = self.fdt
        AF = mybir.ActivationFunctionType
        DR = mybir.MatmulPerfMode.DoubleRow

        with (
            tc.tile_pool(name="pin", bufs=1) as pin,
            tc.tile_pool(name="mid", bufs=1) as mid,
            tc.tile_pool(name="prodp", bufs=2) as prodp,
            tc.tile_pool(name="dramp", bufs=1, space="DRAM") as dramp,
        ):
            # phase-nested pools: all three close after P5, then the P6
            # pool reuses their space
            attn_cm = tc.tile_pool(name="attnp", bufs=1)
            attnp = attn_cm.__enter__()
            seq_cm = tc.tile_pool(name="seqp", bufs=1)
            seqp = seq_cm.__enter__()
            wpin_cm = tc.tile_pool(name="wpin", bufs=1)
            wpin = wpin_cm.__enter__()

            # ---------- tiles ----------
            ident = pin.tile([128, 128], fdt)
            ssum = pin.tile([M, E], fdt)
            eadd = pin.tile([E, 1], F32)
            epsb = pin.tile([128, 2], F32)
            ohx = pin.tile([E, RPC], fdt)
            ohy = pin.tile([E, RPC], fdt)
            bh = pin.tile([128, EMB // 128], F32)
            bt = pin.tile([128, EMB // 128], F32)
            bbc = pin.tile([NCL, 1], F32)
            repm = pin.tile([E, 32, 128], fdt)
            ent = mid.tile([M, H], F32)
            if P3_FP8:
                attn = attnp.tile([64, 2, 8, NH, 128], FP8)
                ohxy2 = pin.tile([64, 2, 2 * RPC], FP8)
                av = d["attn"].ap().rearrange(
                    "p (t lc h f) -> p t lc h f", t=2, lc=8, h=NH)
                ov = d["ohxy2"].ap().rearrange("p (t n) -> p t n", t=2)
            else:
                attn = attnp.tile([M, 8, NH, 128], fdt)
                ohxy2 = pin.tile([M, 2 * RPC], fdt)
                av = d["attn"].ap().rearrange(
                    "p (lc h f) -> p lc h f", lc=8, h=NH)
                ov = d["ohxy2"].ap()
            sq = seqp.tile([128, 8, L + 1], fdt)
            wh_sb = wpin.tile([128, 16, EMB], fdt, name="wh_sb")
            wt_sb = wpin.tile([128, 16, EMB], fdt, name="wt_sb")

            # ---------- DMA priority order ----------
            nc.sync.dma_start(ent[:], d["ent"].ap())
            for t, key in [(ssum, "ssum"), (eadd, "eadd")]:
                nc.sync.dma_start(t[:], d[key].ap())
            nc.sync.dma_start(ohxy2[:], ov)
            for t, key in [
                (ident, "ident"), (ohx, "ohx"), (ohy, "ohy"), (epsb, "epsb"),
                (bh, "bh"), (bt, "bt"), (bbc, "bbc"),
            ]:
                nc.sync.dma_start(t[:], d[key].ap())
            if P3_FP8:
                for lc in range(8):
                    nc.sync.dma_start(attn[:, :, lc], av[:, :, lc])
            else:
                for lc in range(8):
                    nc.sync.dma_start(attn[:, lc], av[:, lc])
            nc.sync.dma_start(
                wh_sb[:], d["wh"].ap().rearrange("p (a b) -> p a b", a=16))
            nc.sync.dma_start(
                wt_sb[:], d["wt"].ap().rearrange("p (a b) -> p a b", a=16))
            nc.sync.dma_start(sq[:], d["seq"].ap()
                              .rearrange("p (a b) -> p a b", a=8))
            nc.scalar.dma_start(repm[:], d["repm"].ap()
                                .rearrange("p (a b) -> p a b", a=32))

            # wb stream: prefetch first 4 chunks now (scalar queue) into
            # persistent tiles; the rest rotate through the P6 pool
            wb_tiles = {}
            for k in range(4):
                wb = mid.tile([128, 32 * NCL], fdt, name=f"wbp{k}")
                nc.sync.dma_start(
                    wb[:], d["wb"].ap()[:, k * 32 * NCL:(k + 1) * 32 * NCL])
                wb_tiles[k] = wb

            # ---------- P3 + P1 ----------
            # P1's matmuls are issued between lc0 and lc1 so the PE queue
            # never stalls on the (slow to start) ACT exp; ps_ent shares
            # the P3 PSUM pool to avoid a pool barrier.
            pexp = mid.tile([M, H], fdt, name="pexp")
            nc.scalar.activation(pexp[:], ent[:], AF.Exp)
            ent_sb = mid.tile([E, H], fdt)

            CTmm = mid.tile([128, 8, RPC], fdt, name="CTmm")
            psP_cm = tc.tile_pool(name="psP", bufs=1, space="PSUM")
            psP = psP_cm.__enter__()
            ps_ent = psP.tile([E, H], F32, tag="ent", bufs=1, name="ps_ent")
            for lc in range(8):
                prods = []
                for q in range(8):
                    psq = psP.tile([128, 2, 2, RPC], F32, tag="p3", bufs=3,
                                   name="psq")
                    for hh in range(2):
                        h = q * 2 + hh
                        if P3_FP8:
                            self.mm(psq[:, hh], attn[:, :, lc, h, :],
                                    ohxy2[:], perf_mode=DR)
                        else:
                            self.mm(psq[:, hh], attn[:, lc, h, :], ohxy2[:])
                    prod = prodp.tile([128, 2, RPC], fdt, tag=f"prod{q % 4}",
                                      bufs=4, name=f"prod{q % 4}")
                    route = P3_ROUTES[q]
                    if route in ("b", "g"):
                        g16 = prodp.tile([128, 2, 2, RPC], fdt, tag="g16",
                                         bufs=2, name="g16")
                        nc.scalar.copy(g16[:], psq[:])
                        eng = nc.gpsimd if route == "g" else nc.vector
                        eng.tensor_mul(prod[:], g16[:, :, 0, :],
                                       g16[:, :, 1, :])
                    else:
                        gx = prodp.tile([128, 2, RPC], fdt, tag="gx",
                                        bufs=3, name="gx")
                        nc.scalar.copy(gx[:], psq[:, :, 0, :])
                        nc.vector.tensor_mul(prod[:], gx[:], psq[:, :, 1, :])
                    prods.append(prod)
                if lc == 0:
                    # P1 segment-sum matmuls ride the PE queue here
                    for nh in range(2):
                        self.mm(ps_ent[:, nh * 512:(nh + 1) * 512], ssum[:],
                                pexp[:, nh * 512:(nh + 1) * 512])
                    nc.scalar.activation(ent_sb[:], ps_ent[:], AF.Ln,
                                         bias=eadd[:])
                # tree reduce 8 tiles -> CTmm[:, lc, :]
                for st in (0, 4):
                    nc.vector.tensor_add(prods[st][:], prods[st][:],
                                         prods[st + 1][:])
                    nc.vector.tensor_add(prods[st + 2][:], prods[st + 2][:],
                                         prods[st + 3][:])
                    nc.vector.tensor_add(prods[st][:], prods[st][:],
                                         prods[st + 2][:])
                nc.vector.tensor_add(prods[0][:], prods[0][:], prods[4][:])
                nc.vector.tensor_add(CTmm[:, lc, :], prods[0][:, 0, :],
                                     prods[0][:, 1, :])
            psP_cm.__exit__(None, None, None)

            # entT: [h-part, hc, e] (issued after P3 so the PE queue never
            # stalls on the ident DMA)
            psT_cm = tc.tile_pool(name="psT", bufs=2, space="PSUM")
            psT = psT_cm.__enter__()
            entT = mid.tile([128, 8, E], fdt, name="entT")
            for hc in range(8):
                ps_t2 = psT.tile([128, E], fdt, tag="tp")
                self.tp(ps_t2[:], ent_sb[:, hc * 128:(hc + 1) * 128],
                        ident[0:E, 0:E])
                nc.scalar.copy(entT[:, hc, :], ps_t2[:])
            psT_cm.__exit__(None, None, None)

            # ---------- EW = ent_sb @ W[0:1024] (fills the P3->P4 gap) ----
            psEW_cm = tc.tile_pool(name="psEW", bufs=2, space="PSUM")
            psEW = psEW_cm.__enter__()
            EWh = mid.tile([E, EMB], fdt, name="EWh")
            EWt = mid.tile([E, EMB], fdt, name="EWt")
            for w, ew in ((wh_sb, EWh), (wt_sb, EWt)):
                ps_ew = psEW.tile([E, EMB], F32, tag="ew")
                for hc in range(8):
                    for lo, hi in ((0, 512), (512, 768)):
                        self.mm(ps_ew[:, lo:hi], entT[:, hc, :],
                                w[:, hc, lo:hi],
                                start=(hc == 0), stop=(hc == 7))
                nc.scalar.copy(ew[:], ps_ew[:])
            psEW_cm.__exit__(None, None, None)

            # ---------- P4: rel = normalize(C) @ seq ----------
            psR_cm = tc.tile_pool(name="psR", bufs=1, space="PSUM")
            psR = psR_cm.__enter__()
            ps_rel = [psR.tile([128, L], F32, name=f"ps_rel{i}")
                      for i in range(2)]
            ps_s8 = psR.tile([128, 2, 8], F32, name="ps_s8")
            for lc in range(8):
                st, sp = lc == 0, lc == 7
                for rc in range(2):
                    lhsT = CTmm[:, lc, rc * 128:(rc + 1) * 128]
                    self.mm(ps_rel[rc][:, 0:512], lhsT, sq[:, lc, 0:512],
                            start=st, stop=sp)
                    self.mm(ps_rel[rc][:, 512:1024], lhsT, sq[:, lc, 512:1024],
                            start=st, stop=sp)
                    self.mm(ps_s8[:, rc, lc:lc + 1], lhsT,
                            sq[:, lc, 1024:1025], start=True, stop=True)
            relT = mid.tile([128, 8, RPC], fdt, name="relT")
            psT2_cm = tc.tile_pool(name="psT2", bufs=2, space="PSUM")
            psT2 = psT2_cm.__enter__()
            for rc in range(2):
                tdenom = prodp.tile([128, 1], F32, tag="tden")
                nc.vector.tensor_reduce(tdenom[:], ps_s8[:, rc, :],
                                        axis=mybir.AxisListType.X,
                                        op=mybir.AluOpType.add)
                nc.scalar.activation(tdenom[:], tdenom[:], AF.Identity,
                                     bias=epsb[:, rc:rc + 1], scale=1.0)
                frec = prodp.tile([128, 1], F32, tag="frec")
                nc.vector.reciprocal(frec[:], tdenom[:])
                rel_sc = mid.tile([128, L], fdt, tag="rel_sc", name="rel_sc")
                nc.vector.tensor_scalar_mul(rel_sc[:], ps_rel[rc][:], frec[:])
                for dc in range(8):
                    ps_t = psT2.tile([128, 128], fdt, tag="tp2")
                    self.tp(ps_t[:], rel_sc[:, dc * 128:(dc + 1) * 128],
                            ident[:])
                    nc.scalar.copy(relT[:, dc, rc * 128:(rc + 1) * 128],
                                   ps_t[:])
            psT2_cm.__exit__(None, None, None)
            psR_cm.__exit__(None, None, None)

            # ---------- P5: extractors -> hsEt/tsEt [emb, n] ----------
            psE_cm = tc.tile_pool(name="psE", bufs=4, space="PSUM")
            psE = psE_cm.__enter__()
            hsEt = mid.tile([128, 6, RPC], fdt, name="hsEt")
            tsEt = mid.tile([128, 6, RPC], fdt, name="tsEt")
            # ec-major staging: the P6 broadcast reads one contiguous 16KB
            # block per partition
            hsd = dramp.tile([6, 128, RPC], fdt, name="hsd")
            tsd = dramp.tile([6, 128, RPC], fdt, name="tsd")
            for ec in range(6):
                for (w, bvec, ew, oh, dst, dstd) in (
                    (wh_sb, bh, EWh, ohx, hsEt, hsd),
                    (wt_sb, bt, EWt, ohy, tsEt, tsd),
                ):
                    ps_e = psE.tile([128, RPC], F32, tag="pe", name="ps_e")
                    self.mm(ps_e[:], ew[:, ec * 128:(ec + 1) * 128], oh[:],
                            start=True, stop=False)
                    for kc in range(8, 16):
                        self.mm(ps_e[:], w[:, kc, ec * 128:(ec + 1) * 128],
                                relT[:, kc % 8, :],
                                start=False, stop=(kc == 15))
                    nc.scalar.activation(dst[:, ec, :], ps_e[:], AF.Tanh,
                                         bias=bvec[:, ec:ec + 1])
                    nc.sync.dma_start(dstd[ec], dst[:, ec, :])
            psE_cm.__exit__(None, None, None)
            wpin_cm.__exit__(None, None, None)
            seq_cm.__exit__(None, None, None)
            attn_cm.__exit__(None, None, None)

            # ---------- P6: block bilinear + classifier ----------
            with (
                tc.tile_pool(name="blph", bufs=1) as blph,
                tc.tile_pool(name="ps_lt", bufs=1, space="PSUM") as ps_lt,
                tc.tile_pool(name="psRep", bufs=2, space="PSUM") as psRep,
            ):
                pslt = ps_lt.tile([NCL, RPC], F32)

                def issue_b2t(k, tag="b2t", bufs=3):
                    kk = 64 * (k % 2)
                    ec = k // 2
                    b2t = blph.tile([128, RPC], fdt, tag=tag, bufs=bufs,
                                    name=tag)
                    for h0 in (0, 1):
                        nc.sync.dma_start(b2t[64 * h0:64 * (h0 + 1)],
                                          tsd[ec, kk:kk + 64, :])
                    return b2t

                def issue_b1rep(k, bufs=3, tag="b1rep"):
                    # split the two replication halves across both hardware
                    # DGE rings — one ring sustains only ~150-180 GB/s
                    kk = 64 * (k % 2)
                    ec = k // 2
                    b1rep = blph.tile([128, 32, RPC], fdt, tag=tag, bufs=bufs,
                                      name=tag)
                    for h0, eng in ((0, nc.sync), (1, nc.scalar)):
                        src = hsd[ec, kk + 32 * h0:kk + 32 * (h0 + 1), :] \
                            .unsqueeze(0).broadcast_to([64, 32, RPC])
                        eng.dma_start(b1rep[64 * h0:64 * (h0 + 1)], src)
                    return b1rep

                cg = 0
                for k in range(K12):
                    kk = 64 * (k % 2)
                    ec = k // 2
                    route = P6_ROUTES[k]
                    if k in wb_tiles:
                        wb = wb_tiles[k]
                    else:
                        wb = blph.tile([128, 32 * NCL], fdt, tag="wb",
                                       bufs=3, name="wb")
                        nc.sync.dma_start(
                            wb[:],
                            d["wb"].ap()[:, k * 32 * NCL:(k + 1) * 32 * NCL])
                    b2t = issue_b2t(k)
                    blT = blph.tile([128, 32, RPC], fdt, tag="blT",
                                    bufs=2, name="blT")
                    if route in ("pd", "pa"):
                        assert kk == 0, "PE replication route needs even k"
                        hsE64 = hsEt[kk:kk + 64, ec, :]
                        b2b = b2t[:].unsqueeze(1).broadcast_to([128, 4, RPC])
                        for cq in range(8):
                            psq6 = psRep.tile([128, 4, RPC], F32, tag="rep",
                                              bufs=2, name="psq6")
                            for i4 in range(4):
                                self.mm(psq6[:, i4, :],
                                        repm[:, cq * 4 + i4, :], hsE64)
                            if route == "pa":
                                b1c = blph.tile([128, 4, RPC], fdt, tag="b1c",
                                                bufs=3, name="b1c")
                                nc.scalar.copy(b1c[:], psq6[:])
                                nc.vector.tensor_mul(
                                    blT[:, cq * 4:(cq + 1) * 4, :],
                                    b1c[:], b2b)
                            else:
                                nc.vector.tensor_mul(
                                    blT[:, cq * 4:(cq + 1) * 4, :],
                                    psq6[:], b2b)
                    else:
                        b1rep = issue_b1rep(k)
                        b2b = b2t[:].unsqueeze(1).broadcast_to([128, 8, RPC])
                        for g in range(4):
                            nc.vector.tensor_mul(
                                blT[:, g * 8:(g + 1) * 8, :],
                                b1rep[:, g * 8:(g + 1) * 8, :], b2b)
                    for c in range(32):
                        self.mm(pslt[:], wb[:, c * NCL:(c + 1) * NCL],
                                blT[:, c, :],
                                start=(cg == 0), stop=(cg == 383))
                        cg += 1

                out_sb = mid.tile([NCL, RPC], F32)
                nc.scalar.activation(out_sb[:], pslt[:], AF.Identity,
                                     bias=bbc[:])
                nc.sync.dma_start(d["lt"].ap(), out_sb[:])


_PROGRAM_CACHE = {}


def _get_program():
    if _FULL_KEY not in _PROGRAM_CACHE:
        _PROGRAM_CACHE[_FULL_KEY] = _Builder()
    return _PROGRAM_CACHE[_FULL_KEY]


def _host_inputs(seq_lhs, ent_lhs, ent_to_seq_attn, entity_id_labels, hts,
                 Wh, bh, Wt, bt, Wb, bb):
    """Build the 8 per-core input maps (all host-side numpy)."""
    fdt = _np_fdt()
    seq_lhs = np.asarray(seq_lhs, np.float32)
    ent_lhs = np.asarray(ent_lhs, np.float32)
    ent_to_seq_attn = np.asarray(ent_to_seq_attn, np.float32)
    entity_id_labels = np.asarray(entity_id_labels)
    hts = np.asarray(hts)
    Wh = np.asarray(Wh, np.float32)
    Wt = np.asarray(Wt, np.float32)
    Wb = np.asarray(Wb, np.float32)
    bh = np.asarray(bh, np.float32)
    bt = np.asarray(bt, np.float32)
    bb = np.asarray(bb, np.float32)

    # device chunk (k, c) row p maps to Wb row k*4096 + i*64 + j with
    # i = c + 32*(p//64), j = p%64
    p_ = np.arange(128)
    c_ = np.arange(32)
    k_ = np.arange(K12)
    rows = (k_[:, None, None] * 4096
            + (c_[None, :, None] + 32 * (p_[None, None, :] // 64)) * 64
            + (p_[None, None, :] % 64))  # [k, c, p]
    wb_r = np.ascontiguousarray(
        Wb[rows.reshape(-1), :].reshape(K12 * 32, 128, NCL)
        .transpose(1, 0, 2).reshape(128, 384 * NCL)
    ).astype(fdt)
    wh_c = np.ascontiguousarray(
        Wh.reshape(16, 128, EMB).transpose(1, 0, 2).reshape(128, 16 * EMB)
    ).astype(fdt)
    wt_c = np.ascontiguousarray(
        Wt.reshape(16, 128, EMB).transpose(1, 0, 2).reshape(128, 16 * EMB)
    ).astype(fdt)
    bh_c = np.ascontiguousarray(bh.reshape(EMB // 128, 128).T)
    bt_c = np.ascontiguousarray(bt.reshape(EMB // 128, 128).T)
    bb_c = np.ascontiguousarray(bb.reshape(NCL, 1))
    ident = np.eye(128, dtype=np.float32).astype(fdt)
    # repm[r, c, p] = 1 iff r == c + 32*(p//64)
    repm_h = np.zeros((E, 32, 128), np.float32)
    for c in range(32):
        repm_h[c, c, 0:64] = 1.0
        repm_h[c + 32, c, 64:128] = 1.0
    repm_h = repm_h.reshape(E, 32 * 128).astype(fdt)

    in_maps = []
    for c in range(NCORES):
        doc, half = divmod(c, 2)
        sl = slice(half * RPC, (half + 1) * RPC)
        labels = entity_id_labels[doc].astype(np.int64)
        cnt = np.bincount(labels, minlength=E).astype(np.float32)
        S = np.zeros((M, E), np.float32)
        S[np.arange(M), labels] = 1.0
        eadd = (cnt == 0).astype(np.float32).reshape(E, 1)
        hi = hts[doc, sl, 0].astype(np.int64)
        ti = hts[doc, sl, 1].astype(np.int64)
        ohx = np.zeros((E, RPC), np.float32)
        ohx[hi, np.arange(RPC)] = 1.0
        ohy = np.zeros((E, RPC), np.float32)
        ohy[ti, np.arange(RPC)] = 1.0
        # exact-scaled gather: x columns are 0/1, y columns are 0 or 1/64
        # (both exact in fp8/fp16); the per-pair 1/(cntx*cnty) mean factor
        # cancels in the rel normalization, with the 1e-5 eps rescaled by
        # epsb = 16e-5 * cntx * cnty / 64 per pair
        ohxs = (S @ ohx)            # [M, RPC] in {0, 1}
        ohys = (S @ ohy) * (1.0 / 64.0)
        cnt1 = np.maximum(cnt, 1.0)
        scl = cnt1[hi] * cnt1[ti] / 64.0    # [RPC]
        epsb = np.ascontiguousarray(
            (16e-5 * scl).reshape(2, 128).T.astype(np.float32))  # [128, 2]
        ohxy2 = np.concatenate([ohxs, ohys], axis=1)  # [M, 512]
        a = (ent_to_seq_attn[doc].transpose(1, 0, 2)  # [M, NH, L]
             .reshape(M, NH, 8, 128).transpose(0, 2, 1, 3))  # [M, 8, NH, 128]
        if P3_FP8:
            f8 = _np_fp8()
            # m = t*64 + k  ->  [k(64), t(2), lc, h, f]
            attn_c = np.ascontiguousarray(
                a.reshape(2, 64, 8, NH, 128).transpose(1, 0, 2, 3, 4)
                .reshape(64, 2 * 8 * NH * 128)).astype(f8)
            ohxy2_c = np.ascontiguousarray(
                ohxy2.reshape(2, 64, 2 * RPC).transpose(1, 0, 2)
                .reshape(64, 2 * 2 * RPC)).astype(f8)
        else:
            attn_c = np.ascontiguousarray(
                a.reshape(M, 8 * NH * 128)).astype(fdt)
            ohxy2_c = ohxy2.astype(fdt)
        seq_r = seq_lhs[doc].reshape(8, 128, L).transpose(1, 0, 2)
        seq_aug = np.concatenate(
            [seq_r, np.ones((128, 8, 1), np.float32)], axis=2
        )
        in_maps.append({
            "ent": np.ascontiguousarray(ent_lhs[doc]),
            "attn": attn_c,
            "seq": np.ascontiguousarray(
                seq_aug.reshape(128, 8 * (L + 1))).astype(fdt),
            "ssum": S.astype(fdt),
            "ohxy2": ohxy2_c,
            "epsb": epsb,
            "eadd": eadd,
            "ohx": ohx.astype(fdt),
            "ohy": ohy.astype(fdt),
            "wh": wh_c, "wt": wt_c, "bh": bh_c, "bt": bt_c,
            "wb": wb_r, "bbc": bb_c, "ident": ident, "repm": repm_h,
        })
    return in_maps


_LAST_RESULTS = {}


def kernel(**inputs) -> np.ndarray:
    prog = _get_program()
    in_maps = _host_inputs(**inputs)
    trace = os.environ.get("DOCRED_TRACE", "0") == "1"
    res = run_bass_kernel_spmd(
        prog.nc, in_maps, core_ids=list(range(NCORES)), trace=trace,
    )
    _LAST_RESULTS["res"] = res
    out = np.empty((B * R, NCL), np.float32)
    for c in range(NCORES):
        doc, half = divmod(c, 2)
        lt = res.results[c]["lt"]  # [NCL, RPC]
        out[doc * R + half * RPC: doc * R + (half + 1) * RPC, :] = lt.T
    return out
